# revision 9
# baseline (speedup 1.0000x reference)
"""Trainium2 8-core tensor-parallel attention kernel (Bass/Tile).

Full inputs in, full output out. Sharding: tensor-parallel over heads
(4 heads per core). Fused single-pipeline schedule:
  proj(b0) -> attn(b0) || proj(b1) -> attn(b1) || o_proj(b0) -> o_proj(b1)
with chunked AllGathers (one per (batch, 512-query block)) overlapped
with compute. q/k stay in SBUF; v round-trips DRAM in attention layout.
Causal structure exploited at 128-column granularity on the diagonal.
"""
import sys

for _p in ("/opt/trn_rl_repo",):
    if _p not in sys.path:
        sys.path.insert(0, _p)

import numpy as np
import ml_dtypes

import concourse.bass as bass
import concourse.mybir as mybir
import concourse.tile as tile
from concourse import bacc
from concourse.bass_utils import run_bass_kernel_spmd

B, S, D, H = 2, 2048, 4096, 32
HD = D // H          # 128 head dim
T = B * S            # 4096 tokens
NC = 8               # cores
HL = H // NC         # 4 heads per core
DH = HL * HD         # 512 dims per core
SCALE = 1.0 / float(np.sqrt(HD))
BF16 = mybir.dt.bfloat16
F32 = mybir.dt.float32
bf16 = ml_dtypes.bfloat16

NT = T // 512        # 8 token slices of 512
NSB = S // 512       # 4 slices per batch
NCT = D // 128       # 32 contraction tiles
NKB = S // 128       # 16 key blocks per batch

_CACHE = {}
LAST_RESULT = None


def build():
    nc = bacc.Bacc("TRN2", target_bir_lowering=False, debug=False, num_devices=NC)

    xT = nc.dram_tensor("xT", [D, T], BF16, kind="ExternalInput").ap()
    wqT = nc.dram_tensor("wqT", [D, DH], BF16, kind="ExternalInput").ap()
    wkT = nc.dram_tensor("wkT", [D, DH], BF16, kind="ExternalInput").ap()
    wvT = nc.dram_tensor("wvT", [D, DH], BF16, kind="ExternalInput").ap()
    woT = nc.dram_tensor("woT", [D, DH], BF16, kind="ExternalInput").ap()
    cosE = nc.dram_tensor("cosE", [HD, S], BF16, kind="ExternalInput").ap()
    sinE = nc.dram_tensor("sinE", [HD, S], BF16, kind="ExternalInput").ap()
    rotT = nc.dram_tensor("rotT", [HD, HD], BF16, kind="ExternalInput").ap()
    mask128 = nc.dram_tensor("mask128", [128, 128], F32, kind="ExternalInput").ap()
    ones128 = nc.dram_tensor("ones128", [128, 1], BF16, kind="ExternalInput").ap()
    ones1f = nc.dram_tensor("ones1f", [1, 128], F32, kind="ExternalInput").ap()
    out = nc.dram_tensor("out", [T, DH], F32, kind="ExternalOutput").ap()

    with tile.TileContext(nc) as tc:
        with tc.tile_pool(name="dram", bufs=1, space="DRAM") as dram, \
             tc.tile_pool(name="cons", bufs=1) as cons, \
             tc.tile_pool(name="qk", bufs=1) as qkp, \
             tc.tile_pool(name="vh", bufs=4) as vhp, \
             tc.tile_pool(name="aw", bufs=1) as aw, \
             tc.tile_pool(name="aps", bufs=1, space="PSUM") as aps:

            # ---- DRAM internals ----
            vd = {}
            for b in range(B):
                for h in range(HL):
                    vd[(b, h)] = dram.tile([128, NKB * 128], BF16, name=f"vd{b}{h}")
            agin = {}
            agout = {}
            for b in range(B):
                for jq in range(NSB):
                    agin[(b, jq)] = dram.tile([DH, 512], BF16, name=f"agin{b}{jq}")
                    agout[(b, jq)] = dram.tile([NC * DH, 512], BF16,
                                               addr_space="Shared", name=f"agout{b}{jq}")

            # ---- constants ----
            rot_sb = cons.tile([128, 128], BF16, name="rot_sb")
            nc.sync.dma_start(rot_sb[:], rotT[:])
            mask_sb = cons.tile([128, 128], F32, name="mask_sb")
            nc.sync.dma_start(mask_sb[:], mask128[:])
            o128_sb = cons.tile([128, 1], BF16, name="o128_sb")
            nc.sync.dma_start(o128_sb[:], ones128[:])
            o1f_sb = cons.tile([1, 128], F32, name="o1f_sb")
            nc.sync.dma_start(o1f_sb[:], ones1f[:])

            # ---- long-lived SBUF: q/k per (b,head), attention working ----
            q_sb, k_sb = {}, {}
            for b in range(B):
                for h in range(HL):
                    q_sb[(b, h)] = qkp.tile([128, S], BF16, tag=f"q{b}{h}", name=f"qsb{b}{h}")
                    k_sb[(b, h)] = qkp.tile([128, S], BF16, tag=f"k{b}{h}", name=f"ksb{b}{h}")

            def proj_batch(pw, px, prw, pps, b):
                """Projections + rope for batch b (token slices 4b..4b+3)."""
                # weight tiles for q/k live across the whole phase (resident)
                for t in range(NSB * b, NSB * (b + 1)):
                    tok = t * 512
                    off = (t % NSB) * 512
                    cos_t = prw.tile([128, 512], BF16, tag="cos", name=f"cos{t}")
                    sin_t = prw.tile([128, 512], BF16, tag="sin", name=f"sin{t}")
                    nc.sync.dma_start(cos_t[:], cosE[:, off:off + 512])
                    nc.sync.dma_start(sin_t[:], sinE[:, off:off + 512])
                    xt = []
                    for c in range(NCT):
                        xc = px.tile([128, 512], BF16, tag="xt", bufs=33, name=f"x{t}_{c}")
                        nc.sync.dma_start(xc[:], xT[c * 128:(c + 1) * 128, tok:tok + 512])
                        xt.append(xc)
                    # q/k projections (dim-major) + rope -> SBUF
                    for wname, dst in (("q", q_sb), ("k", k_sb)):
                        for i in range(HL):
                            ps = pps.tile([128, 512], F32, tag="pp", bufs=3, name=f"ps{wname}{t}{i}")
                            for c in range(NCT):
                                nc.tensor.matmul(
                                    ps[:], pw[(wname, c)][:, i * 128:(i + 1) * 128],
                                    xt[c][:], start=(c == 0), stop=(c == NCT - 1))
                            qsb = prw.tile([128, 512], BF16, tag="qsb", name=f"qq{wname}{t}{i}")
                            nc.vector.tensor_copy(qsb[:], ps[:])
                            pr = pps.tile([128, 512], F32, tag="pp", bufs=3, name=f"pr{wname}{t}{i}")
                            nc.tensor.matmul(pr[:], rot_sb[:], qsb[:], start=True, stop=True)
                            qc = prw.tile([128, 512], BF16, tag="qc", name=f"qc{wname}{t}{i}")
                            nc.vector.tensor_tensor(qc[:], ps[:], cos_t[:], mybir.AluOpType.mult)
                            qr = prw.tile([128, 512], BF16, tag="qr", name=f"qr{wname}{t}{i}")
                            nc.vector.tensor_tensor(qr[:], pr[:], sin_t[:], mybir.AluOpType.mult)
                            nc.vector.tensor_tensor(
                                dst[(b, i)][:, off:off + 512], qc[:], qr[:],
                                mybir.AluOpType.add)
                    # v projection (token-major), tt pairs; wv streamed
                    for half in range(2):
                        psv = []
                        for u in range(2):
                            p = pps.tile([128, 512], F32, tag="pp", bufs=3, name=f"psv{t}{half}{u}")
                            psv.append(p)
                        for c in range(NCT):
                            wv_c = px.tile([128, 512], BF16, tag="wv", bufs=4, name=f"wv{t}{half}{c}")
                            nc.sync.dma_start(wv_c[:], wvT[c * 128:(c + 1) * 128, :])
                            for u in range(2):
                                tt = half * 2 + u
                                nc.tensor.matmul(
                                    psv[u][:], xt[c][:, tt * 128:(tt + 1) * 128],
                                    wv_c[:], start=(c == 0), stop=(c == NCT - 1))
                        for u in range(2):
                            tt = half * 2 + u
                            kb = (t % NSB) * 4 + tt
                            vst = prw.tile([128, 512], BF16, tag="vst", bufs=2, name=f"vs{t}{half}{u}")
                            nc.vector.tensor_copy(vst[:], psv[u][:])
                            for h in range(HL):
                                nc.sync.dma_start(
                                    vd[(b, h)][:, kb * 128:(kb + 1) * 128],
                                    vst[:, h * 128:(h + 1) * 128])

            def attn_batch(b):
                """Attention for batch b; AllGather per (b, jq) chunk."""
                vh = {}
                for h in range(HL):
                    vh[h] = vhp.tile([128, NKB * 128], BF16, tag="vh", name=f"vh{b}{h}")
                    nc.sync.dma_start(vh[h][:], vd[(b, h)][:])
                for jq in range(NSB):
                    for h in range(HL):
                        qh, kh = q_sb[(b, h)], k_sb[(b, h)]
                        acc = aps.tile([128, 512], F32, tag="acc", bufs=2, name=f"acc{b}{jq}{h}")
                        sums = aps.tile([1, 512], F32, tag="sums", bufs=1, name=f"sums{b}{jq}{h}")
                        nkt = 4 * (jq + 1)
                        for kt in range(nkt):
                            diag = kt >= 4 * jq
                            m = kt - 4 * jq
                            qoff = jq * 512 + (m * 128 if diag else 0)
                            n = 512 - (m * 128 if diag else 0)
                            ro = qoff - jq * 512      # offset within acc/sums
                            pss = aps.tile([128, 512], F32, tag="pss", bufs=2, name=f"pss{b}{jq}{h}{kt}")
                            nc.tensor.matmul(
                                pss[:, :n], kh[:, kt * 128:(kt + 1) * 128],
                                qh[:, qoff:jq * 512 + 512], start=True, stop=True)
                            if diag:
                                nc.vector.tensor_tensor(
                                    pss[:, :128], pss[:, :128], mask_sb[:],
                                    mybir.AluOpType.add)
                            ex = aw.tile([128, 512], BF16, tag="ex", bufs=3, name=f"ex{b}{jq}{h}{kt}")
                            nc.scalar.activation(ex[:, :n], pss[:, :n],
                                                 mybir.ActivationFunctionType.Exp, scale=SCALE)
                            nc.tensor.matmul(acc[:, ro:512], vh[h][:, kt * 128:(kt + 1) * 128],
                                             ex[:, :n], start=(kt == 0), stop=(kt == nkt - 1))
                            nc.tensor.matmul(sums[:, ro:512], o128_sb[:], ex[:, :n],
                                             start=(kt == 0), stop=(kt == nkt - 1))
                        rec = aw.tile([1, 512], F32, tag="rec", bufs=2, name=f"rec{b}{jq}{h}")
                        nc.vector.reciprocal(rec[:], sums[:])
                        rb = aps.tile([128, 512], F32, tag="pss", bufs=2, name=f"rb{b}{jq}{h}")
                        nc.tensor.matmul(rb[:], o1f_sb[:], rec[:], start=True, stop=True)
                        rbs = aw.tile([128, 512], F32, tag="rbs", bufs=2, name=f"rbs{b}{jq}{h}")
                        nc.vector.tensor_copy(rbs[:], rb[:])
                        att = aw.tile([128, 512], BF16, tag="att", bufs=2, name=f"att{b}{jq}{h}")
                        nc.vector.tensor_tensor(att[:], acc[:], rbs[:], mybir.AluOpType.mult)
                        nc.sync.dma_start(agin[(b, jq)][h * 128:(h + 1) * 128, :], att[:])
                    nc.gpsimd.collective_compute(
                        "AllGather", mybir.AluOpType.bypass,
                        replica_groups=[list(range(NC))],
                        ins=[agin[(b, jq)].opt()], outs=[agout[(b, jq)].opt()])

            # ================= emission =================
            with tc.tile_pool(name="pw", bufs=1) as pwp, \
                 tc.tile_pool(name="px", bufs=1) as px, \
                 tc.tile_pool(name="prw", bufs=2) as prw, \
                 tc.tile_pool(name="pps", bufs=1, space="PSUM") as pps:
                pw = {}
                for wname, w_dr in (("q", wqT), ("k", wkT)):
                    for c in range(NCT):
                        wt = pwp.tile([128, DH], BF16, name=f"w{wname}_{c}")
                        nc.sync.dma_start(wt[:], w_dr[c * 128:(c + 1) * 128, :])
                        pw[(wname, c)] = wt
                proj_batch(pw, px, prw, pps, 0)
                attn_batch(0)
                proj_batch(pw, px, prw, pps, 1)
            # projection pools released here
            attn_batch(1)

            # ---------------- o_proj ----------------
            with tc.tile_pool(name="ores", bufs=1) as ores, \
                 tc.tile_pool(name="och", bufs=3) as och, \
                 tc.tile_pool(name="oo", bufs=4) as oo, \
                 tc.tile_pool(name="ops", bufs=3, space="PSUM") as ops:
                wo_sb = ores.tile([128, NCT * DH], BF16, name="wo_sb")
                nc.sync.dma_start(
                    wo_sb[:].rearrange("p (c i) -> p c i", c=NCT),
                    woT.rearrange("(c p) i -> p c i", p=128))
                for b in range(B):
                    for jq in range(NSB):
                        for tt in range(4):
                            ch = och.tile([128, NCT * 128], BF16, tag="ch", name=f"ch{b}{jq}{tt}")
                            nc.sync.dma_start(
                                ch[:].rearrange("p (c u) -> p c u", c=NCT),
                                agout[(b, jq)].rearrange("(c p) t -> p c t", p=128)[:, :, tt * 128:(tt + 1) * 128])
                            pso = ops.tile([128, 512], F32, tag="pso", name=f"pso{b}{jq}{tt}")
                            for i in range(NCT):
                                nc.tensor.matmul(pso[:], ch[:, i * 128:(i + 1) * 128],
                                                 wo_sb[:, i * DH:(i + 1) * DH],
                                                 start=(i == 0), stop=(i == NCT - 1))
                            ot = oo.tile([128, 512], F32, tag="ot", name=f"ot{b}{jq}{tt}")
                            nc.vector.tensor_copy(ot[:], pso[:])
                            row = b * S + jq * 512 + tt * 128
                            nc.sync.dma_start(out[row:row + 128, :], ot[:])

    nc.compile()
    return nc


def _host_prep(x, freqs_cos, freqs_sin, mask, wq, wk, wv, wo):
    xT = np.ascontiguousarray(x.reshape(T, D).T).astype(bf16)
    cos = np.asarray(freqs_cos, np.float32)   # [S, 64]
    sin = np.asarray(freqs_sin, np.float32)
    cosE = np.ascontiguousarray(np.repeat(cos.T, 2, axis=0)).astype(bf16)  # [128, S]
    sinE = np.ascontiguousarray(np.repeat(sin.T, 2, axis=0)).astype(bf16)
    rot = np.zeros((HD, HD), np.float32)
    idx = np.arange(0, HD, 2)
    rot[idx, idx + 1] = -1.0                  # rot(q)[2d] = -q[2d+1]
    rot[idx + 1, idx] = 1.0                   # rot(q)[2d+1] = q[2d]
    rotT = np.ascontiguousarray(rot.T).astype(bf16)
    # causal triangle for a 128x128 diagonal block, pre-scaled for exp(scale*x)
    kk = np.arange(128)
    mask128 = np.where(kk[:, None] > kk[None, :], -1e9 / SCALE, 0.0).astype(np.float32)
    ones128 = np.ones((128, 1), bf16)
    ones1f = np.ones((1, 128), np.float32)
    shared = dict(xT=xT, cosE=cosE, sinE=sinE, rotT=rotT, mask128=mask128,
                  ones128=ones128, ones1f=ones1f)
    in_maps = []
    for r in range(NC):
        sl = slice(r * DH, (r + 1) * DH)
        m = dict(shared)
        m["wqT"] = np.ascontiguousarray(np.asarray(wq, np.float32)[sl, :].T).astype(bf16)
        m["wkT"] = np.ascontiguousarray(np.asarray(wk, np.float32)[sl, :].T).astype(bf16)
        m["wvT"] = np.ascontiguousarray(np.asarray(wv, np.float32)[sl, :].T).astype(bf16)
        m["woT"] = np.ascontiguousarray(np.asarray(wo, np.float32)[sl, :].T).astype(bf16)
        in_maps.append(m)
    return in_maps


def kernel(x, freqs_cos, freqs_sin, mask, wq, wk, wv, wo, start_pos):
    global LAST_RESULT
    if "nc" not in _CACHE:
        _CACHE["nc"] = build()
    nc = _CACHE["nc"]
    in_maps = _host_prep(x, freqs_cos, freqs_sin, mask, wq, wk, wv, wo)
    res = run_bass_kernel_spmd(nc, in_maps, core_ids=list(range(NC)))
    LAST_RESULT = res
    parts = [res.results[r]["out"] for r in range(NC)]
    full = np.concatenate(parts, axis=1)      # [T, D]
    return np.ascontiguousarray(full.reshape(B, S, D)).astype(np.float32)


# revision 10
# speedup vs baseline: 1.2055x; 1.2055x over previous
"""Trainium2 8-core tensor-parallel attention kernel (Bass/Tile).

Full inputs in, full output out. Tensor-parallel over heads (4 per core).
Fused schedule: proj(b0) -> attn(b0) || proj(b1) -> attn(b1) || o_proj,
with one AllGather per (batch, 512-query chunk) overlapped with compute.
Causal structure exploited at 128-column granularity on the diagonal.
All projection weights SBUF-resident; x streamed per 512-token slice;
q/k/v round-trip DRAM in attention-friendly layouts.
"""
import sys

for _p in ("/opt/trn_rl_repo",):
    if _p not in sys.path:
        sys.path.insert(0, _p)

import numpy as np
import ml_dtypes

import concourse.bass as bass
import concourse.mybir as mybir
import concourse.tile as tile
from concourse import bacc
from concourse.bass_utils import run_bass_kernel_spmd

B, S, D, H = 2, 2048, 4096, 32
HD = D // H          # 128 head dim
T = B * S            # 4096 tokens
NC = 8               # cores
HL = H // NC         # 4 heads per core
DH = HL * HD         # 512 dims per core
SCALE = 1.0 / float(np.sqrt(HD))
BF16 = mybir.dt.bfloat16
F32 = mybir.dt.float32
bf16 = ml_dtypes.bfloat16

NT = T // 512        # 8 token slices of 512
NSB = S // 512       # 4 slices per batch
NCT = D // 128       # 32 contraction tiles
NKB = S // 128       # 16 key blocks per batch

_CACHE = {}
LAST_RESULT = None


def build():
    nc = bacc.Bacc("TRN2", target_bir_lowering=False, debug=False, num_devices=NC)

    xT = nc.dram_tensor("xT", [D, T], BF16, kind="ExternalInput").ap()
    wqT = nc.dram_tensor("wqT", [D, DH], BF16, kind="ExternalInput").ap()
    wkT = nc.dram_tensor("wkT", [D, DH], BF16, kind="ExternalInput").ap()
    wvT = nc.dram_tensor("wvT", [D, DH], BF16, kind="ExternalInput").ap()
    woT = nc.dram_tensor("woT", [D, DH], BF16, kind="ExternalInput").ap()
    cosE = nc.dram_tensor("cosE", [HD, S], BF16, kind="ExternalInput").ap()
    sinE = nc.dram_tensor("sinE", [HD, S], BF16, kind="ExternalInput").ap()
    rotT = nc.dram_tensor("rotT", [HD, HD], BF16, kind="ExternalInput").ap()
    mask128 = nc.dram_tensor("mask128", [128, 128], F32, kind="ExternalInput").ap()
    ones128 = nc.dram_tensor("ones128", [128, 1], BF16, kind="ExternalInput").ap()
    out = nc.dram_tensor("out", [T, DH], F32, kind="ExternalOutput").ap()

    with tile.TileContext(nc) as tc:
        with tc.tile_pool(name="dram", bufs=1, space="DRAM") as dram, \
             tc.tile_pool(name="cons", bufs=1) as cons, \
             tc.tile_pool(name="qkh", bufs=1) as qkh, \
             tc.tile_pool(name="vh", bufs=5) as vhp, \
             tc.tile_pool(name="aw", bufs=1) as aw, \
             tc.tile_pool(name="aps", bufs=1, space="PSUM") as aps:

            # ---- DRAM internals ----
            qd, kd, vd = {}, {}, {}
            for b in range(B):
                for h in range(HL):
                    qd[(b, h)] = dram.tile([128, S], BF16, name=f"qd{b}{h}")
                    kd[(b, h)] = dram.tile([128, S], BF16, name=f"kd{b}{h}")
                    vd[(b, h)] = dram.tile([128, NKB * 128], BF16, name=f"vd{b}{h}")
            agin, agout = {}, {}
            for b in range(B):
                for jq in range(NSB):
                    agin[(b, jq)] = dram.tile([DH, 512], BF16, name=f"agin{b}{jq}")
                    agout[(b, jq)] = dram.tile([NC * DH, 512], BF16,
                                               addr_space="Shared", name=f"agout{b}{jq}")

            # ---- constants ----
            rot_sb = cons.tile([128, 128], BF16, name="rot_sb")
            nc.sync.dma_start(rot_sb[:], rotT[:])
            mask_sb = cons.tile([128, 128], F32, name="mask_sb")
            nc.sync.dma_start(mask_sb[:], mask128[:])
            o128_sb = cons.tile([128, 1], BF16, name="o128_sb")
            nc.sync.dma_start(o128_sb[:], ones128[:])

            def load_x(px, t):
                tok = t * 512
                xt = []
                for c in range(NCT):
                    xc = px.tile([128, 512], BF16, tag="xt", bufs=32, name=f"x{t}_{c}")
                    nc.sync.dma_start(xc[:], xT[c * 128:(c + 1) * 128, tok:tok + 512])
                    xt.append(xc)
                return xt

            def proj_batch(pw, px, prw, pps, b, xt_first):
                for t in range(NSB * b, NSB * (b + 1)):
                    off = (t % NSB) * 512
                    cos_t = prw.tile([128, 512], BF16, tag="cos", name=f"cos{t}")
                    sin_t = prw.tile([128, 512], BF16, tag="sin", name=f"sin{t}")
                    nc.sync.dma_start(cos_t[:], cosE[:, off:off + 512])
                    nc.sync.dma_start(sin_t[:], sinE[:, off:off + 512])
                    xt = xt_first if t == NSB * b and xt_first is not None else load_x(px, t)
                    # q/k projections (dim-major) + rope -> DRAM
                    for wname, dst in (("q", qd), ("k", kd)):
                        for i in range(HL):
                            ps = pps.tile([128, 512], F32, tag="pp", bufs=3, name=f"ps{wname}{t}{i}")
                            for c in range(NCT):
                                nc.tensor.matmul(
                                    ps[:], pw[(wname, c)][:, i * 128:(i + 1) * 128],
                                    xt[c][:], start=(c == 0), stop=(c == NCT - 1))
                            qsb = prw.tile([128, 512], BF16, tag="qsb", name=f"qq{wname}{t}{i}")
                            nc.vector.tensor_copy(qsb[:], ps[:])
                            pr = pps.tile([128, 512], F32, tag="pp", bufs=3, name=f"pr{wname}{t}{i}")
                            nc.tensor.matmul(pr[:], rot_sb[:], qsb[:], start=True, stop=True)
                            qc = prw.tile([128, 512], BF16, tag="qc", name=f"qc{wname}{t}{i}")
                            nc.vector.tensor_tensor(qc[:], ps[:], cos_t[:], mybir.AluOpType.mult)
                            qr = prw.tile([128, 512], BF16, tag="qr", name=f"qr{wname}{t}{i}")
                            nc.vector.tensor_tensor(qr[:], pr[:], sin_t[:], mybir.AluOpType.mult)
                            qf = prw.tile([128, 512], BF16, tag="qf", name=f"qf{wname}{t}{i}")
                            nc.vector.tensor_tensor(qf[:], qc[:], qr[:], mybir.AluOpType.add)
                            nc.sync.dma_start(dst[(b, i)][:, off:off + 512], qf[:])
                    # v projection (token-major), tt-pairs, wv resident
                    for half in range(2):
                        psv = []
                        for u in range(2):
                            p = pps.tile([128, 512], F32, tag="pp", bufs=3, name=f"psv{t}{half}{u}")
                            psv.append(p)
                        for c in range(NCT):
                            for u in range(2):
                                tt = half * 2 + u
                                nc.tensor.matmul(
                                    psv[u][:], xt[c][:, tt * 128:(tt + 1) * 128],
                                    pw[("v", c)][:], start=(c == 0), stop=(c == NCT - 1))
                        for u in range(2):
                            tt = half * 2 + u
                            kb = (t % NSB) * 4 + tt
                            vst = prw.tile([128, 512], BF16, tag="vst", bufs=2, name=f"vs{t}{half}{u}")
                            nc.vector.tensor_copy(vst[:], psv[u][:])
                            for h in range(HL):
                                nc.sync.dma_start(
                                    vd[(b, h)][:, kb * 128:(kb + 1) * 128],
                                    vst[:, h * 128:(h + 1) * 128])

            def attn_batch(b):
                vh, qh, kh = {}, {}, {}
                for h in range(HL):
                    qh[h] = qkh.tile([128, S], BF16, tag="qh", bufs=4, name=f"qh{b}{h}")
                    nc.sync.dma_start(qh[h][:], qd[(b, h)][:])
                    kh[h] = qkh.tile([128, S], BF16, tag="kh", bufs=4, name=f"kh{b}{h}")
                    nc.sync.dma_start(kh[h][:], kd[(b, h)][:])
                    vh[h] = vhp.tile([128, NKB * 128], BF16, tag="vh", name=f"vh{b}{h}")
                    nc.sync.dma_start(vh[h][:], vd[(b, h)][:])
                for jq in range(NSB):
                    for h in range(HL):
                        acc = aps.tile([128, 512], F32, tag="acc", bufs=2, name=f"acc{b}{jq}{h}")
                        sums = aps.tile([1, 512], F32, tag="sums", bufs=1, name=f"sums{b}{jq}{h}")
                        nkt = 4 * (jq + 1)
                        for kt in range(nkt):
                            diag = kt >= 4 * jq
                            m = kt - 4 * jq
                            qoff = jq * 512 + (m * 128 if diag else 0)
                            n = 512 - (m * 128 if diag else 0)
                            ro = qoff - jq * 512      # offset within acc/sums
                            pss = aps.tile([128, 512], F32, tag="pss", bufs=2, name=f"pss{b}{jq}{h}{kt}")
                            nc.tensor.matmul(
                                pss[:, :n], kh[h][:, kt * 128:(kt + 1) * 128],
                                qh[h][:, qoff:jq * 512 + 512], start=True, stop=True)
                            if diag:
                                nc.vector.tensor_tensor(
                                    pss[:, :128], pss[:, :128], mask_sb[:],
                                    mybir.AluOpType.add)
                            ex = aw.tile([128, 512], BF16, tag="ex", bufs=3, name=f"ex{b}{jq}{h}{kt}")
                            nc.scalar.activation(ex[:, :n], pss[:, :n],
                                                 mybir.ActivationFunctionType.Exp, scale=SCALE)
                            nc.tensor.matmul(acc[:, ro:512], vh[h][:, kt * 128:(kt + 1) * 128],
                                             ex[:, :n], start=(kt == 0), stop=(kt == nkt - 1))
                            nc.tensor.matmul(sums[:, ro:512], o128_sb[:], ex[:, :n],
                                             start=(kt == 0), stop=(kt == nkt - 1))
                        rec = aw.tile([1, 512], F32, tag="rec", bufs=2, name=f"rec{b}{jq}{h}")
                        nc.vector.reciprocal(rec[:], sums[:])
                        rbs = aw.tile([128, 512], F32, tag="rbs", bufs=2, name=f"rbs{b}{jq}{h}")
                        nc.gpsimd.partition_broadcast(rbs[:], rec[:])
                        att = aw.tile([128, 512], BF16, tag="att", bufs=2, name=f"att{b}{jq}{h}")
                        nc.vector.tensor_tensor(att[:], acc[:], rbs[:], mybir.AluOpType.mult)
                        nc.sync.dma_start(agin[(b, jq)][h * 128:(h + 1) * 128, :], att[:])
                    nc.gpsimd.collective_compute(
                        "AllGather", mybir.AluOpType.bypass,
                        replica_groups=[list(range(NC))],
                        ins=[agin[(b, jq)].opt()], outs=[agout[(b, jq)].opt()])

            # ================= emission =================
            with tc.tile_pool(name="pw", bufs=1) as pwp, \
                 tc.tile_pool(name="px", bufs=1) as px, \
                 tc.tile_pool(name="prw", bufs=2) as prw, \
                 tc.tile_pool(name="pps", bufs=1, space="PSUM") as pps:
                xt0 = load_x(px, 0)                 # x slice 0 first: PE starts early
                pw = {}
                for wname, w_dr in (("q", wqT), ("k", wkT), ("v", wvT)):
                    for c in range(NCT):
                        wt = pwp.tile([128, DH], BF16, name=f"w{wname}_{c}")
                        nc.sync.dma_start(wt[:], w_dr[c * 128:(c + 1) * 128, :])
                        pw[(wname, c)] = wt
                proj_batch(pw, px, prw, pps, 0, xt0)
                attn_batch(0)
                proj_batch(pw, px, prw, pps, 1, None)
            # projection pools (weights, x, rope work, proj psum) released here
            attn_batch(1)

            # ---------------- o_proj ----------------
            with tc.tile_pool(name="ores", bufs=1) as ores, \
                 tc.tile_pool(name="och", bufs=3) as och, \
                 tc.tile_pool(name="oo", bufs=4) as oo, \
                 tc.tile_pool(name="ops", bufs=3, space="PSUM") as ops:
                wo_sb = ores.tile([128, NCT * DH], BF16, name="wo_sb")
                nc.sync.dma_start(
                    wo_sb[:].rearrange("p (c i) -> p c i", c=NCT),
                    woT.rearrange("(c p) i -> p c i", p=128))
                for b in range(B):
                    for jq in range(NSB):
                        for tt in range(4):
                            ch = och.tile([128, NCT * 128], BF16, tag="ch", name=f"ch{b}{jq}{tt}")
                            nc.sync.dma_start(
                                ch[:].rearrange("p (c u) -> p c u", c=NCT),
                                agout[(b, jq)].rearrange("(c p) t -> p c t", p=128)[:, :, tt * 128:(tt + 1) * 128])
                            pso = ops.tile([128, 512], F32, tag="pso", name=f"pso{b}{jq}{tt}")
                            for i in range(NCT):
                                nc.tensor.matmul(pso[:], ch[:, i * 128:(i + 1) * 128],
                                                 wo_sb[:, i * DH:(i + 1) * DH],
                                                 start=(i == 0), stop=(i == NCT - 1))
                            ot = oo.tile([128, 512], F32, tag="ot", name=f"ot{b}{jq}{tt}")
                            nc.vector.tensor_copy(ot[:], pso[:])
                            row = b * S + jq * 512 + tt * 128
                            nc.sync.dma_start(out[row:row + 128, :], ot[:])

    nc.compile()
    return nc


def _host_prep(x, freqs_cos, freqs_sin, mask, wq, wk, wv, wo):
    xT = np.ascontiguousarray(x.reshape(T, D).T).astype(bf16)
    cos = np.asarray(freqs_cos, np.float32)   # [S, 64]
    sin = np.asarray(freqs_sin, np.float32)
    cosE = np.ascontiguousarray(np.repeat(cos.T, 2, axis=0)).astype(bf16)  # [128, S]
    sinE = np.ascontiguousarray(np.repeat(sin.T, 2, axis=0)).astype(bf16)
    rot = np.zeros((HD, HD), np.float32)
    idx = np.arange(0, HD, 2)
    rot[idx, idx + 1] = -1.0                  # rot(q)[2d] = -q[2d+1]
    rot[idx + 1, idx] = 1.0                   # rot(q)[2d+1] = q[2d]
    rotT = np.ascontiguousarray(rot.T).astype(bf16)
    # causal triangle for a 128x128 diagonal block, pre-scaled for exp(scale*x)
    kk = np.arange(128)
    mask128 = np.where(kk[:, None] > kk[None, :], -1e9 / SCALE, 0.0).astype(np.float32)
    ones128 = np.ones((128, 1), bf16)
    shared = dict(xT=xT, cosE=cosE, sinE=sinE, rotT=rotT, mask128=mask128,
                  ones128=ones128)
    in_maps = []
    for r in range(NC):
        sl = slice(r * DH, (r + 1) * DH)
        m = dict(shared)
        m["wqT"] = np.ascontiguousarray(np.asarray(wq, np.float32)[sl, :].T).astype(bf16)
        m["wkT"] = np.ascontiguousarray(np.asarray(wk, np.float32)[sl, :].T).astype(bf16)
        m["wvT"] = np.ascontiguousarray(np.asarray(wv, np.float32)[sl, :].T).astype(bf16)
        m["woT"] = np.ascontiguousarray(np.asarray(wo, np.float32)[sl, :].T).astype(bf16)
        in_maps.append(m)
    return in_maps


def kernel(x, freqs_cos, freqs_sin, mask, wq, wk, wv, wo, start_pos):
    global LAST_RESULT
    if "nc" not in _CACHE:
        _CACHE["nc"] = build()
    nc = _CACHE["nc"]
    in_maps = _host_prep(x, freqs_cos, freqs_sin, mask, wq, wk, wv, wo)
    res = run_bass_kernel_spmd(nc, in_maps, core_ids=list(range(NC)))
    LAST_RESULT = res
    parts = [res.results[r]["out"] for r in range(NC)]
    full = np.concatenate(parts, axis=1)      # [T, D]
    return np.ascontiguousarray(full.reshape(B, S, D)).astype(np.float32)


# revision 20
# speedup vs baseline: 1.3783x; 1.1433x over previous
"""Trainium2 8-core tensor-parallel attention kernel (Bass/Tile).

Full inputs in, full output out. Tensor-parallel over heads (4 per core).
Fused schedule: proj(b0) -> attn(b0) || proj(b1) -> attn(b1) || o_proj,
with one AllGather per (batch, 512-query chunk) overlapped with compute.
Causal structure exploited at 128-column granularity on the diagonal.
All projection weights SBUF-resident; x streamed per 512-token slice;
q/k/v round-trip DRAM in attention-friendly layouts.
"""
import sys

for _p in ("/opt/trn_rl_repo",):
    if _p not in sys.path:
        sys.path.insert(0, _p)

import numpy as np
import ml_dtypes

import concourse.bass as bass
import concourse.mybir as mybir
import concourse.tile as tile
from concourse import bacc
from concourse.bass_utils import run_bass_kernel_spmd

B, S, D, H = 2, 2048, 4096, 32
HD = D // H          # 128 head dim
T = B * S            # 4096 tokens
NC = 8               # cores
HL = H // NC         # 4 heads per core
DH = HL * HD         # 512 dims per core
SCALE = 1.0 / float(np.sqrt(HD))
BF16 = mybir.dt.bfloat16
F32 = mybir.dt.float32
bf16 = ml_dtypes.bfloat16

NT = T // 512        # 8 token slices of 512
NSB = S // 512       # 4 slices per batch
NCT = D // 128       # 32 contraction tiles
NKB = S // 128       # 16 key blocks per batch

_CACHE = {}
LAST_RESULT = None


def build():
    nc = bacc.Bacc("TRN2", target_bir_lowering=False, debug=False, num_devices=NC)

    xT = nc.dram_tensor("xT", [D, T], BF16, kind="ExternalInput").ap()
    wqT = nc.dram_tensor("wqT", [D, DH], BF16, kind="ExternalInput").ap()
    wkT = nc.dram_tensor("wkT", [D, DH], BF16, kind="ExternalInput").ap()
    wvT = nc.dram_tensor("wvT", [D, DH], BF16, kind="ExternalInput").ap()
    woT = nc.dram_tensor("woT", [D, DH], BF16, kind="ExternalInput").ap()
    cosE = nc.dram_tensor("cosE", [HD, S], BF16, kind="ExternalInput").ap()
    sinE = nc.dram_tensor("sinE", [HD, S], BF16, kind="ExternalInput").ap()
    rotT = nc.dram_tensor("rotT", [HD, HD], BF16, kind="ExternalInput").ap()
    mask128 = nc.dram_tensor("mask128", [128, 128], F32, kind="ExternalInput").ap()
    ones128 = nc.dram_tensor("ones128", [128, 1], BF16, kind="ExternalInput").ap()
    out = nc.dram_tensor("out", [T, DH], F32, kind="ExternalOutput").ap()

    with tile.TileContext(nc) as tc:
        with tc.tile_pool(name="dram", bufs=1, space="DRAM") as dram, \
             tc.tile_pool(name="cons", bufs=1) as cons, \
             tc.tile_pool(name="qkh", bufs=1) as qkh, \
             tc.tile_pool(name="vh", bufs=5) as vhp, \
             tc.tile_pool(name="aw", bufs=1) as aw, \
             tc.tile_pool(name="aps", bufs=1, space="PSUM") as aps:

            # ---- DRAM internals ----
            qd, kd, vd = {}, {}, {}
            for b in range(B):
                for h in range(HL):
                    qd[(b, h)] = dram.tile([128, S], BF16, name=f"qd{b}{h}")
                    kd[(b, h)] = dram.tile([128, S], BF16, name=f"kd{b}{h}")
                    vd[(b, h)] = dram.tile([128, NKB * 128], BF16, name=f"vd{b}{h}")
            agin, agout = {}, {}
            for b in range(B):
                for jq in range(NSB):
                    agin[(b, jq)] = dram.tile([DH, 512], BF16, name=f"agin{b}{jq}")
                    agout[(b, jq)] = dram.tile([NC * DH, 512], BF16,
                                               addr_space="Shared", name=f"agout{b}{jq}")

            # ---- constants ----
            rot_sb = cons.tile([128, 128], BF16, name="rot_sb")
            nc.sync.dma_start(rot_sb[:], rotT[:])
            mask_sb = cons.tile([128, 128], F32, name="mask_sb")
            nc.sync.dma_start(mask_sb[:], mask128[:])
            o128_sb = cons.tile([128, 1], BF16, name="o128_sb")
            nc.sync.dma_start(o128_sb[:], ones128[:])

            def load_x(px, t):
                tok = t * 512
                xt = []
                for c in range(NCT):
                    xc = px.tile([128, 512], BF16, tag="xt", bufs=32, name=f"x{t}_{c}")
                    nc.sync.dma_start(xc[:], xT[c * 128:(c + 1) * 128, tok:tok + 512])
                    xt.append(xc)
                return xt

            def proj_batch(pw, px, prw, pps, b, xt_first):
                for t in range(NSB * b, NSB * (b + 1)):
                    off = (t % NSB) * 512
                    cos_t = prw.tile([128, 512], BF16, tag="cos", name=f"cos{t}")
                    sin_t = prw.tile([128, 512], BF16, tag="sin", name=f"sin{t}")
                    nc.sync.dma_start(cos_t[:], cosE[:, off:off + 512])
                    nc.sync.dma_start(sin_t[:], sinE[:, off:off + 512])
                    xt = xt_first if t == NSB * b and xt_first is not None else load_x(px, t)
                    # q/k projections (dim-major) + rope -> DRAM
                    for wname, dst in (("q", qd), ("k", kd)):
                        for i in range(HL):
                            ps = pps.tile([128, 512], F32, tag="pp", bufs=2, name=f"ps{wname}{t}{i}")
                            for c in range(NCT):
                                nc.tensor.matmul(
                                    ps[:], pw[(wname, c)][:, i * 128:(i + 1) * 128],
                                    xt[c][:], start=(c == 0), stop=(c == NCT - 1))
                            qsb = prw.tile([128, 512], BF16, tag="qsb", name=f"qq{wname}{t}{i}")
                            nc.any.tensor_copy(qsb[:], ps[:])
                            pr = aps.tile([128, 512], F32, tag="pss", bufs=3, name=f"pr{wname}{t}{i}")
                            nc.tensor.matmul(pr[:], rot_sb[:], qsb[:], start=True, stop=True)
                            qc = prw.tile([128, 512], BF16, tag="qc", name=f"qc{wname}{t}{i}")
                            nc.vector.tensor_tensor(qc[:], ps[:], cos_t[:], mybir.AluOpType.mult)
                            qr = prw.tile([128, 512], BF16, tag="qr", name=f"qr{wname}{t}{i}")
                            nc.vector.tensor_tensor(qr[:], pr[:], sin_t[:], mybir.AluOpType.mult)
                            qf = prw.tile([128, 512], BF16, tag="qf", name=f"qf{wname}{t}{i}")
                            nc.vector.tensor_tensor(qf[:], qc[:], qr[:], mybir.AluOpType.add)
                            nc.sync.dma_start(dst[(b, i)][:, off:off + 512], qf[:])
                    # v projection (token-major), tt-pairs, wv resident
                    for half in range(2):
                        psv = []
                        for u in range(2):
                            p = pps.tile([128, 512], F32, tag="pp", bufs=2, name=f"psv{t}{half}{u}")
                            psv.append(p)
                        for c in range(NCT):
                            for u in range(2):
                                tt = half * 2 + u
                                nc.tensor.matmul(
                                    psv[u][:], xt[c][:, tt * 128:(tt + 1) * 128],
                                    pw[("v", c)][:], start=(c == 0), stop=(c == NCT - 1))
                        for u in range(2):
                            tt = half * 2 + u
                            kb = (t % NSB) * 4 + tt
                            vst = prw.tile([128, 512], BF16, tag="vst", bufs=2, name=f"vs{t}{half}{u}")
                            nc.any.tensor_copy(vst[:], psv[u][:])
                            for h in range(HL):
                                nc.sync.dma_start(
                                    vd[(b, h)][:, kb * 128:(kb + 1) * 128],
                                    vst[:, h * 128:(h + 1) * 128])

            def attn_batch(b):
                vh, qh, kh = {}, {}, {}
                for h in range(HL):
                    qh[h] = qkh.tile([128, S], BF16, tag="qh", bufs=4, name=f"qh{b}{h}")
                    kh[h] = qkh.tile([128, S], BF16, tag="kh", bufs=4, name=f"kh{b}{h}")
                    vh[h] = vhp.tile([128, NKB * 128], BF16, tag="vh", name=f"vh{b}{h}")
                # chunked loads: chunk jq only needs proj slice (b, jq) done,
                # so attention overlaps the tail of this batch's projections
                for jq in range(NSB):
                    sl = slice(jq * 512, (jq + 1) * 512)
                    for h in range(HL):
                        nc.sync.dma_start(qh[h][:, sl], qd[(b, h)][:, sl])
                        nc.sync.dma_start(kh[h][:, sl], kd[(b, h)][:, sl])
                        nc.sync.dma_start(vh[h][:, sl], vd[(b, h)][:, sl])
                for jq in range(NSB):
                    for h in range(HL):
                        acc = aps.tile([128, 512], F32, tag="acc", bufs=2, name=f"acc{b}{jq}{h}")
                        sums = aps.tile([1, 512], F32, tag="sums", bufs=1, name=f"sums{b}{jq}{h}")
                        nkt = 4 * (jq + 1)
                        for kt in range(nkt):
                            diag = kt >= 4 * jq
                            m = kt - 4 * jq
                            qoff = jq * 512 + (m * 128 if diag else 0)
                            n = 512 - (m * 128 if diag else 0)
                            ro = qoff - jq * 512      # offset within acc/sums
                            pss = aps.tile([128, 512], F32, tag="pss", bufs=3, name=f"pss{b}{jq}{h}{kt}")
                            nc.tensor.matmul(
                                pss[:, :n], kh[h][:, kt * 128:(kt + 1) * 128],
                                qh[h][:, qoff:jq * 512 + 512], start=True, stop=True)
                            if diag:
                                nc.vector.tensor_tensor(
                                    pss[:, :128], pss[:, :128], mask_sb[:],
                                    mybir.AluOpType.add)
                            ex = aw.tile([128, 512], BF16, tag="ex", bufs=4, name=f"ex{b}{jq}{h}{kt}")
                            nc.scalar.activation(ex[:, :n], pss[:, :n],
                                                 mybir.ActivationFunctionType.Exp, scale=SCALE)
                            nc.tensor.matmul(acc[:, ro:512], vh[h][:, kt * 128:(kt + 1) * 128],
                                             ex[:, :n], start=(kt == 0), stop=(kt == nkt - 1))
                            nc.tensor.matmul(sums[:, ro:512], o128_sb[:], ex[:, :n],
                                             start=(kt == 0), stop=(kt == nkt - 1))
                        rec = aw.tile([1, 512], F32, tag="rec", bufs=1, name=f"rec{b}{jq}{h}")
                        nc.vector.reciprocal(rec[:], sums[:])
                        rbs = aw.tile([128, 512], F32, tag="rbs", bufs=2, name=f"rbs{b}{jq}{h}")
                        nc.gpsimd.partition_broadcast(rbs[:], rec[:])
                        att = aw.tile([128, 512], BF16, tag="att", bufs=2, name=f"att{b}{jq}{h}")
                        nc.vector.tensor_tensor(att[:], acc[:], rbs[:], mybir.AluOpType.mult)
                        nc.sync.dma_start(agin[(b, jq)][h * 128:(h + 1) * 128, :], att[:])
                    nc.gpsimd.collective_compute(
                        "AllGather", mybir.AluOpType.bypass,
                        replica_groups=[list(range(NC))],
                        ins=[agin[(b, jq)].opt()], outs=[agout[(b, jq)].opt()])

            # ================= emission =================
            with tc.tile_pool(name="pw", bufs=1) as pwp, \
                 tc.tile_pool(name="px", bufs=1) as px, \
                 tc.tile_pool(name="prw", bufs=2) as prw, \
                 tc.tile_pool(name="pps", bufs=1, space="PSUM") as pps:
                # interleave x slice 0 with wq so the first psum group's
                # dependencies land pairwise and the PE starts ~immediately
                pw = {}
                xt0 = []
                for c in range(NCT):
                    xc = px.tile([128, 512], BF16, tag="xt", bufs=32, name=f"x0_{c}")
                    nc.sync.dma_start(xc[:], xT[c * 128:(c + 1) * 128, 0:512])
                    xt0.append(xc)
                    wt = pwp.tile([128, DH], BF16, name=f"wq_{c}")
                    nc.sync.dma_start(wt[:], wqT[c * 128:(c + 1) * 128, :])
                    pw[("q", c)] = wt
                for wname, w_dr in (("k", wkT), ("v", wvT)):
                    for c in range(NCT):
                        wt = pwp.tile([128, DH], BF16, name=f"w{wname}_{c}")
                        nc.sync.dma_start(wt[:], w_dr[c * 128:(c + 1) * 128, :])
                        pw[(wname, c)] = wt
                proj_batch(pw, px, prw, pps, 0, xt0)
                attn_batch(0)
                proj_batch(pw, px, prw, pps, 1, None)
            # projection pools (weights, x, rope work, proj psum) released here
            attn_batch(1)

            # ---------------- o_proj ----------------
            with tc.tile_pool(name="ores", bufs=1) as ores, \
                 tc.tile_pool(name="och", bufs=3) as och, \
                 tc.tile_pool(name="oo", bufs=4) as oo, \
                 tc.tile_pool(name="ops", bufs=2, space="PSUM") as ops:
                wo_sb = ores.tile([128, NCT * DH], BF16, name="wo_sb")
                nc.sync.dma_start(
                    wo_sb[:].rearrange("p (c i) -> p c i", c=NCT),
                    woT.rearrange("(c p) i -> p c i", p=128))
                for b in range(B):
                    for jq in range(NSB):
                        for tt in range(4):
                            ch = och.tile([128, NCT * 128], BF16, tag="ch", name=f"ch{b}{jq}{tt}")
                            nc.sync.dma_start(
                                ch[:].rearrange("p (c u) -> p c u", c=NCT),
                                agout[(b, jq)].rearrange("(c p) t -> p c t", p=128)[:, :, tt * 128:(tt + 1) * 128])
                            pso = ops.tile([128, 512], F32, tag="pso", name=f"pso{b}{jq}{tt}")
                            for i in range(NCT):
                                nc.tensor.matmul(pso[:], ch[:, i * 128:(i + 1) * 128],
                                                 wo_sb[:, i * DH:(i + 1) * DH],
                                                 start=(i == 0), stop=(i == NCT - 1))
                            ot = oo.tile([128, 512], F32, tag="ot", name=f"ot{b}{jq}{tt}")
                            nc.any.tensor_copy(ot[:], pso[:])
                            row = b * S + jq * 512 + tt * 128
                            nc.sync.dma_start(out[row:row + 128, :], ot[:])

    nc.compile()
    return nc


def _host_prep(x, freqs_cos, freqs_sin, mask, wq, wk, wv, wo):
    xT = np.ascontiguousarray(x.reshape(T, D).T).astype(bf16)
    cos = np.asarray(freqs_cos, np.float32)   # [S, 64]
    sin = np.asarray(freqs_sin, np.float32)
    cosE = np.ascontiguousarray(np.repeat(cos.T, 2, axis=0)).astype(bf16)  # [128, S]
    sinE = np.ascontiguousarray(np.repeat(sin.T, 2, axis=0)).astype(bf16)
    rot = np.zeros((HD, HD), np.float32)
    idx = np.arange(0, HD, 2)
    rot[idx, idx + 1] = -1.0                  # rot(q)[2d] = -q[2d+1]
    rot[idx + 1, idx] = 1.0                   # rot(q)[2d+1] = q[2d]
    rotT = np.ascontiguousarray(rot.T).astype(bf16)
    # causal triangle for a 128x128 diagonal block, pre-scaled for exp(scale*x)
    kk = np.arange(128)
    mask128 = np.where(kk[:, None] > kk[None, :], -1e9 / SCALE, 0.0).astype(np.float32)
    ones128 = np.ones((128, 1), bf16)
    shared = dict(xT=xT, cosE=cosE, sinE=sinE, rotT=rotT, mask128=mask128,
                  ones128=ones128)
    in_maps = []
    for r in range(NC):
        sl = slice(r * DH, (r + 1) * DH)
        m = dict(shared)
        m["wqT"] = np.ascontiguousarray(np.asarray(wq, np.float32)[sl, :].T).astype(bf16)
        m["wkT"] = np.ascontiguousarray(np.asarray(wk, np.float32)[sl, :].T).astype(bf16)
        m["wvT"] = np.ascontiguousarray(np.asarray(wv, np.float32)[sl, :].T).astype(bf16)
        m["woT"] = np.ascontiguousarray(np.asarray(wo, np.float32)[sl, :].T).astype(bf16)
        in_maps.append(m)
    return in_maps


def kernel(x, freqs_cos, freqs_sin, mask, wq, wk, wv, wo, start_pos):
    global LAST_RESULT
    if "nc" not in _CACHE:
        _CACHE["nc"] = build()
    nc = _CACHE["nc"]
    in_maps = _host_prep(x, freqs_cos, freqs_sin, mask, wq, wk, wv, wo)
    res = run_bass_kernel_spmd(nc, in_maps, core_ids=list(range(NC)))
    LAST_RESULT = res
    parts = [res.results[r]["out"] for r in range(NC)]
    full = np.concatenate(parts, axis=1)      # [T, D]
    return np.ascontiguousarray(full.reshape(B, S, D)).astype(np.float32)


# revision 23
# speedup vs baseline: 1.4052x; 1.0195x over previous
"""Trainium2 8-core tensor-parallel attention kernel (Bass/Tile).

Full inputs in, full output out. Tensor-parallel over heads (4 per core).
Fused schedule: proj(b0) -> attn(b0) || proj(b1) -> attn(b1) || o_proj,
with one AllGather per (batch, 512-query chunk) overlapped with compute,
and o_proj chunks emission-interleaved with attention.
Causal structure exploited at 128-column granularity on the diagonal.
All hot DMA paths are fully contiguous (slice-major x on the host,
per-(batch,head,chunk) q/k/v DRAM tiles, per-feature-block o_proj loads).
RoPE rotation done by a partition pair-swap DMA with the sign folded
into sin (no tensor-engine rotation matmuls).
"""
import sys

for _p in ("/opt/trn_rl_repo",):
    if _p not in sys.path:
        sys.path.insert(0, _p)

import numpy as np
import ml_dtypes

import concourse.bass as bass
import concourse.mybir as mybir
import concourse.tile as tile
from concourse import bacc
from concourse.bass_utils import run_bass_kernel_spmd

B, S, D, H = 2, 2048, 4096, 32
HD = D // H          # 128 head dim
T = B * S            # 4096 tokens
NC = 8               # cores
HL = H // NC         # 4 heads per core
DH = HL * HD         # 512 dims per core
SCALE = 1.0 / float(np.sqrt(HD))
BF16 = mybir.dt.bfloat16
F32 = mybir.dt.float32
bf16 = ml_dtypes.bfloat16

NT = T // 512        # 8 token slices of 512
NSB = S // 512       # 4 slices per batch
NCT = D // 128       # 32 contraction tiles
NKB = S // 128       # 16 key blocks per batch

_CACHE = {}
LAST_RESULT = None


def build():
    nc = bacc.Bacc("TRN2", target_bir_lowering=False, debug=False, num_devices=NC)

    # x stored slice-major: [NT, D, 512] so every x tile DMA is contiguous
    xS = nc.dram_tensor("xS", [NT, D, 512], BF16, kind="ExternalInput").ap()
    wqT = nc.dram_tensor("wqT", [D, DH], BF16, kind="ExternalInput").ap()
    wkT = nc.dram_tensor("wkT", [D, DH], BF16, kind="ExternalInput").ap()
    wvT = nc.dram_tensor("wvT", [D, DH], BF16, kind="ExternalInput").ap()
    woT = nc.dram_tensor("woT", [D, DH], BF16, kind="ExternalInput").ap()
    cosE = nc.dram_tensor("cosE", [HD, S], BF16, kind="ExternalInput").ap()
    sinE = nc.dram_tensor("sinE", [HD, S], BF16, kind="ExternalInput").ap()  # sign-folded
    mask128 = nc.dram_tensor("mask128", [128, 128], F32, kind="ExternalInput").ap()
    ones128 = nc.dram_tensor("ones128", [128, 1], BF16, kind="ExternalInput").ap()
    out = nc.dram_tensor("out", [T, DH], F32, kind="ExternalOutput").ap()

    with tile.TileContext(nc) as tc:
        with tc.tile_pool(name="dram", bufs=1, space="DRAM") as dram, \
             tc.tile_pool(name="cons", bufs=1) as cons, \
             tc.tile_pool(name="qkh", bufs=1) as qkh, \
             tc.tile_pool(name="vh", bufs=5) as vhp, \
             tc.tile_pool(name="aw", bufs=1) as aw, \
             tc.tile_pool(name="aps", bufs=1, space="PSUM") as aps:

            # ---- DRAM internals: per (b, head, jq-chunk), all contiguous ----
            qd, kd, vd = {}, {}, {}
            for b in range(B):
                for h in range(HL):
                    for jq in range(NSB):
                        qd[(b, h, jq)] = dram.tile([128, 512], BF16, name=f"qd{b}{h}{jq}")
                        kd[(b, h, jq)] = dram.tile([128, 512], BF16, name=f"kd{b}{h}{jq}")
                        vd[(b, h, jq)] = dram.tile([128, 512], BF16, name=f"vd{b}{h}{jq}")
            agin, agout = {}, {}
            for b in range(B):
                for jq in range(NSB):
                    agin[(b, jq)] = dram.tile([DH, 512], BF16, name=f"agin{b}{jq}")
                    agout[(b, jq)] = dram.tile([NC * DH, 512], BF16,
                                               addr_space="Shared", name=f"agout{b}{jq}")

            # ---- constants ----
            mask_sb = cons.tile([128, 128], F32, name="mask_sb")
            nc.sync.dma_start(mask_sb[:], mask128[:])
            o128_sb = cons.tile([128, 1], BF16, name="o128_sb")
            nc.sync.dma_start(o128_sb[:], ones128[:])

            def load_x(px, t):
                xt = []
                for c in range(NCT):
                    xc = px.tile([128, 512], BF16, tag="xt", bufs=32, name=f"x{t}_{c}")
                    nc.sync.dma_start(xc[:], xS[t, c * 128:(c + 1) * 128, :])
                    xt.append(xc)
                return xt

            def proj_batch(pw, px, prw, pps, b, xt_first):
                for t in range(NSB * b, NSB * (b + 1)):
                    jq = t % NSB
                    off = jq * 512
                    cos_t = prw.tile([128, 512], BF16, tag="cos", name=f"cos{t}")
                    sin_t = prw.tile([128, 512], BF16, tag="sin", name=f"sin{t}")
                    nc.sync.dma_start(cos_t[:], cosE[:, off:off + 512])
                    nc.sync.dma_start(sin_t[:], sinE[:, off:off + 512])
                    xt = xt_first if (t == NSB * b and xt_first is not None) else load_x(px, t)
                    # q/k projections (dim-major) + rope -> DRAM chunk tiles
                    for wname, dst in (("q", qd), ("k", kd)):
                        for i in range(HL):
                            ps = pps.tile([128, 512], F32, tag="pp", bufs=2, name=f"ps{wname}{t}{i}")
                            for c in range(NCT):
                                nc.tensor.matmul(
                                    ps[:], pw[(wname, c)][:, i * 128:(i + 1) * 128],
                                    xt[c][:], start=(c == 0), stop=(c == NCT - 1))
                            qsb = prw.tile([128, 512], BF16, tag="qsb", name=f"qq{wname}{t}{i}")
                            nc.any.tensor_copy(qsb[:], ps[:])
                            # pair-swap via SBUF->SBUF DMA (sign folded into sinE)
                            qsw = prw.tile([128, 512], BF16, tag="qsw", name=f"qw{wname}{t}{i}")
                            qsb_r = qsb.rearrange("(d two) n -> two d n", two=2)
                            qsw_r = qsw.rearrange("(d two) n -> two d n", two=2)
                            nc.sync.dma_start(qsw_r[0], qsb_r[1])
                            nc.sync.dma_start(qsw_r[1], qsb_r[0])
                            qc = prw.tile([128, 512], BF16, tag="qc", name=f"qc{wname}{t}{i}")
                            nc.vector.tensor_tensor(qc[:], ps[:], cos_t[:], mybir.AluOpType.mult)
                            qr = prw.tile([128, 512], BF16, tag="qr", name=f"qr{wname}{t}{i}")
                            nc.vector.tensor_tensor(qr[:], qsw[:], sin_t[:], mybir.AluOpType.mult)
                            qf = prw.tile([128, 512], BF16, tag="qf", name=f"qf{wname}{t}{i}")
                            nc.vector.tensor_tensor(qf[:], qc[:], qr[:], mybir.AluOpType.add)
                            nc.sync.dma_start(dst[(b, i, jq)][:], qf[:])
                    # v projection (token-major), tt-pairs, wv resident
                    for half in range(2):
                        psv = []
                        for u in range(2):
                            p = pps.tile([128, 512], F32, tag="pp", bufs=2, name=f"psv{t}{half}{u}")
                            psv.append(p)
                        for c in range(NCT):
                            for u in range(2):
                                tt = half * 2 + u
                                nc.tensor.matmul(
                                    psv[u][:], xt[c][:, tt * 128:(tt + 1) * 128],
                                    pw[("v", c)][:], start=(c == 0), stop=(c == NCT - 1))
                        for u in range(2):
                            tt = half * 2 + u
                            vst = prw.tile([128, 512], BF16, tag="vst", bufs=2, name=f"vs{t}{half}{u}")
                            nc.any.tensor_copy(vst[:], psv[u][:])
                            for h in range(HL):
                                nc.sync.dma_start(
                                    vd[(b, h, jq)][:, tt * 128:(tt + 1) * 128],
                                    vst[:, h * 128:(h + 1) * 128])

            attn_state = {}

            def attn_setup(b):
                vh, qh, kh = {}, {}, {}
                for h in range(HL):
                    qh[h] = qkh.tile([128, S], BF16, tag="qh", bufs=4, name=f"qh{b}{h}")
                    kh[h] = qkh.tile([128, S], BF16, tag="kh", bufs=4, name=f"kh{b}{h}")
                    vh[h] = vhp.tile([128, NKB * 128], BF16, tag="vh", name=f"vh{b}{h}")
                for jq in range(NSB):
                    sl = slice(jq * 512, (jq + 1) * 512)
                    for h in range(HL):
                        nc.sync.dma_start(qh[h][:, sl], qd[(b, h, jq)][:])
                        nc.sync.dma_start(kh[h][:, sl], kd[(b, h, jq)][:])
                        nc.sync.dma_start(vh[h][:, sl], vd[(b, h, jq)][:])
                attn_state[b] = (qh, kh, vh)

            def attn_chunk(b, jq):
                qh, kh, vh = attn_state[b]
                for h in range(HL):
                    acc = aps.tile([128, 512], F32, tag="acc", bufs=2, name=f"acc{b}{jq}{h}")
                    sums = aps.tile([1, 512], F32, tag="sums", bufs=1, name=f"sums{b}{jq}{h}")
                    nkt = 4 * (jq + 1)
                    for kt in range(nkt):
                        diag = kt >= 4 * jq
                        m = kt - 4 * jq
                        qoff = jq * 512 + (m * 128 if diag else 0)
                        n = 512 - (m * 128 if diag else 0)
                        ro = qoff - jq * 512      # offset within acc/sums
                        pss = aps.tile([128, 512], F32, tag="pss", bufs=3, name=f"pss{b}{jq}{h}{kt}")
                        nc.tensor.matmul(
                            pss[:, :n], kh[h][:, kt * 128:(kt + 1) * 128],
                            qh[h][:, qoff:jq * 512 + 512], start=True, stop=True)
                        if diag:
                            nc.vector.tensor_tensor(
                                pss[:, :128], pss[:, :128], mask_sb[:],
                                mybir.AluOpType.add)
                        ex = aw.tile([128, 512], BF16, tag="ex", bufs=4, name=f"ex{b}{jq}{h}{kt}")
                        nc.scalar.activation(ex[:, :n], pss[:, :n],
                                             mybir.ActivationFunctionType.Exp, scale=SCALE)
                        nc.tensor.matmul(acc[:, ro:512], vh[h][:, kt * 128:(kt + 1) * 128],
                                         ex[:, :n], start=(kt == 0), stop=(kt == nkt - 1))
                        nc.tensor.matmul(sums[:, ro:512], o128_sb[:], ex[:, :n],
                                         start=(kt == 0), stop=(kt == nkt - 1))
                    rec = aw.tile([1, 512], BF16, tag="rec", bufs=1, name=f"rec{b}{jq}{h}")
                    with nc.allow_low_precision(reason="softmax denom reciprocal in bf16 is fine at 2e-2 tol"):
                        nc.vector.reciprocal(rec[:], sums[:])
                    rbs = aw.tile([128, 512], BF16, tag="rbs", bufs=2, name=f"rbs{b}{jq}{h}")
                    nc.gpsimd.partition_broadcast(rbs[:], rec[:])
                    att = aw.tile([128, 512], BF16, tag="att", bufs=2, name=f"att{b}{jq}{h}")
                    nc.vector.tensor_tensor(att[:], acc[:], rbs[:], mybir.AluOpType.mult)
                    nc.sync.dma_start(agin[(b, jq)][h * 128:(h + 1) * 128, :], att[:])
                nc.gpsimd.collective_compute(
                    "AllGather", mybir.AluOpType.bypass,
                    replica_groups=[list(range(NC))],
                    ins=[agin[(b, jq)].opt()], outs=[agout[(b, jq)].opt()])

            # ================= emission =================
            with tc.tile_pool(name="pw", bufs=1) as pwp, \
                 tc.tile_pool(name="px", bufs=1) as px, \
                 tc.tile_pool(name="prw", bufs=2) as prw, \
                 tc.tile_pool(name="pps", bufs=1, space="PSUM") as pps:
                # interleave x slice 0 with wq: first psum group's deps land first
                pw = {}
                xt0 = []
                for c in range(NCT):
                    xc = px.tile([128, 512], BF16, tag="xt", bufs=32, name=f"x0_{c}")
                    nc.sync.dma_start(xc[:], xS[0, c * 128:(c + 1) * 128, :])
                    xt0.append(xc)
                    wt = pwp.tile([128, DH], BF16, name=f"wq_{c}")
                    nc.sync.dma_start(wt[:], wqT[c * 128:(c + 1) * 128, :])
                    pw[("q", c)] = wt
                for wname, w_dr in (("k", wkT), ("v", wvT)):
                    for c in range(NCT):
                        wt = pwp.tile([128, DH], BF16, name=f"w{wname}_{c}")
                        nc.sync.dma_start(wt[:], w_dr[c * 128:(c + 1) * 128, :])
                        pw[(wname, c)] = wt
                proj_batch(pw, px, prw, pps, 0, xt0)
                attn_setup(0)
                for jq in range(NSB):
                    attn_chunk(0, jq)
                proj_batch(pw, px, prw, pps, 1, None)
            # projection pools (weights, x, rope work, proj psum) released here

            with tc.tile_pool(name="ores", bufs=1) as ores, \
                 tc.tile_pool(name="och", bufs=1) as och, \
                 tc.tile_pool(name="oo", bufs=4) as oo, \
                 tc.tile_pool(name="ops", bufs=2, space="PSUM") as ops:
                wo_sb = ores.tile([128, NCT * DH], BF16, name="wo_sb")
                nc.sync.dma_start(
                    wo_sb[:].rearrange("p (c i) -> p c i", c=NCT),
                    woT.rearrange("(c p) i -> p c i", p=128))

                def oproj_chunk(b, jq):
                    # contiguous per-feature-block loads of the gathered chunk
                    ch = []
                    for c in range(NCT):
                        cc = och.tile([128, 512], BF16, tag="ch", bufs=40, name=f"ch{b}{jq}{c}")
                        nc.sync.dma_start(cc[:], agout[(b, jq)][c * 128:(c + 1) * 128, :])
                        ch.append(cc)
                    for tt in range(4):
                        pso = ops.tile([128, 512], F32, tag="pso", name=f"pso{b}{jq}{tt}")
                        for c in range(NCT):
                            nc.tensor.matmul(pso[:], ch[c][:, tt * 128:(tt + 1) * 128],
                                             wo_sb[:, c * DH:(c + 1) * DH],
                                             start=(c == 0), stop=(c == NCT - 1))
                        ot = oo.tile([128, 512], F32, tag="ot", name=f"ot{b}{jq}{tt}")
                        nc.any.tensor_copy(ot[:], pso[:])
                        row = b * S + jq * 512 + tt * 128
                        nc.sync.dma_start(out[row:row + 128, :], ot[:])

                # attention b1 interleaved with o_proj of batch 0's chunks
                attn_setup(1)
                for jq in range(NSB):
                    attn_chunk(1, jq)
                    oproj_chunk(0, jq)
                for jq in range(NSB):
                    oproj_chunk(1, jq)

    nc.compile()
    return nc


def _host_prep(x, freqs_cos, freqs_sin, mask, wq, wk, wv, wo):
    xT = np.asarray(x, np.float32).reshape(T, D).T        # [D, T]
    xSm = np.ascontiguousarray(
        xT.reshape(D, NT, 512).transpose(1, 0, 2)).astype(bf16)   # [NT, D, 512]
    cos = np.asarray(freqs_cos, np.float32)   # [S, 64]
    sin = np.asarray(freqs_sin, np.float32)
    cosE = np.ascontiguousarray(np.repeat(cos.T, 2, axis=0)).astype(bf16)  # [128, S]
    sinE = np.repeat(sin.T, 2, axis=0)                     # [128, S]
    sinE[0::2, :] *= -1.0                                  # sign fold: even rows negative
    sinE = np.ascontiguousarray(sinE).astype(bf16)
    # causal triangle for a 128x128 diagonal block, pre-scaled for exp(scale*x)
    kk = np.arange(128)
    mask128 = np.where(kk[:, None] > kk[None, :], -1e9 / SCALE, 0.0).astype(np.float32)
    ones128 = np.ones((128, 1), bf16)
    shared = dict(xS=xSm, cosE=cosE, sinE=sinE, mask128=mask128, ones128=ones128)
    in_maps = []
    for r in range(NC):
        sl = slice(r * DH, (r + 1) * DH)
        m = dict(shared)
        m["wqT"] = np.ascontiguousarray(np.asarray(wq, np.float32)[sl, :].T).astype(bf16)
        m["wkT"] = np.ascontiguousarray(np.asarray(wk, np.float32)[sl, :].T).astype(bf16)
        m["wvT"] = np.ascontiguousarray(np.asarray(wv, np.float32)[sl, :].T).astype(bf16)
        m["woT"] = np.ascontiguousarray(np.asarray(wo, np.float32)[sl, :].T).astype(bf16)
        in_maps.append(m)
    return in_maps


def kernel(x, freqs_cos, freqs_sin, mask, wq, wk, wv, wo, start_pos):
    global LAST_RESULT
    if "nc" not in _CACHE:
        _CACHE["nc"] = build()
    nc = _CACHE["nc"]
    in_maps = _host_prep(x, freqs_cos, freqs_sin, mask, wq, wk, wv, wo)
    res = run_bass_kernel_spmd(nc, in_maps, core_ids=list(range(NC)))
    LAST_RESULT = res
    parts = [res.results[r]["out"] for r in range(NC)]
    full = np.concatenate(parts, axis=1)      # [T, D]
    return np.ascontiguousarray(full.reshape(B, S, D)).astype(np.float32)


# revision 26
# speedup vs baseline: 1.5330x; 1.0910x over previous
"""Trainium2 8-core tensor-parallel attention kernel (Bass/Tile).

Full inputs in, full output out. Tensor-parallel over heads (4 per core).
Fused schedule: proj(b0) -> attn(b0) || proj(b1) -> attn(b1) || o_proj,
with one AllGather per (batch, 512-query chunk) overlapped with compute,
and o_proj chunks emission-interleaved with attention.
Causal structure exploited at 128-column granularity on the diagonal.
All hot DMA paths are fully contiguous (slice-major x on the host,
per-(batch,head,chunk) q/k/v DRAM tiles, per-feature-block o_proj loads).
RoPE rotation done by a partition pair-swap DMA with the sign folded
into sin (no tensor-engine rotation matmuls).
"""
import sys

for _p in ("/opt/trn_rl_repo",):
    if _p not in sys.path:
        sys.path.insert(0, _p)

import numpy as np
import ml_dtypes

import concourse.bass as bass
import concourse.mybir as mybir
import concourse.tile as tile
from concourse import bacc
from concourse.bass_utils import run_bass_kernel_spmd

B, S, D, H = 2, 2048, 4096, 32
HD = D // H          # 128 head dim
T = B * S            # 4096 tokens
NC = 8               # cores
HL = H // NC         # 4 heads per core
DH = HL * HD         # 512 dims per core
SCALE = 1.0 / float(np.sqrt(HD))
BF16 = mybir.dt.bfloat16
F32 = mybir.dt.float32
bf16 = ml_dtypes.bfloat16

NT = T // 512        # 8 token slices of 512
NSB = S // 512       # 4 slices per batch
NCT = D // 128       # 32 contraction tiles
NKB = S // 128       # 16 key blocks per batch

_CACHE = {}
LAST_RESULT = None


def build():
    nc = bacc.Bacc("TRN2", target_bir_lowering=False, debug=False, num_devices=NC)

    # x stored slice-major: [NT, D, 512] so every x tile DMA is contiguous
    xS = nc.dram_tensor("xS", [NT, D, 512], BF16, kind="ExternalInput").ap()
    wqT = nc.dram_tensor("wqT", [D, DH], BF16, kind="ExternalInput").ap()
    wkT = nc.dram_tensor("wkT", [D, DH], BF16, kind="ExternalInput").ap()
    wvT = nc.dram_tensor("wvT", [D, DH], BF16, kind="ExternalInput").ap()
    woT = nc.dram_tensor("woT", [D, DH], BF16, kind="ExternalInput").ap()
    cosE = nc.dram_tensor("cosE", [HD, S], BF16, kind="ExternalInput").ap()
    sinE = nc.dram_tensor("sinE", [HD, S], BF16, kind="ExternalInput").ap()  # sign-folded
    mask128 = nc.dram_tensor("mask128", [128, 128], F32, kind="ExternalInput").ap()
    ones128 = nc.dram_tensor("ones128", [128, 1], BF16, kind="ExternalInput").ap()
    out = nc.dram_tensor("out", [T, DH], F32, kind="ExternalOutput").ap()

    with tile.TileContext(nc) as tc:
        with tc.tile_pool(name="dram", bufs=1, space="DRAM") as dram, \
             tc.tile_pool(name="cons", bufs=1) as cons, \
             tc.tile_pool(name="qkh", bufs=1) as qkh, \
             tc.tile_pool(name="vh", bufs=5) as vhp, \
             tc.tile_pool(name="aw", bufs=1) as aw, \
             tc.tile_pool(name="aps", bufs=1, space="PSUM") as aps:

            # ---- DRAM internals: per (b, head, jq-chunk), all contiguous ----
            qd, kd, vd = {}, {}, {}
            for b in range(B):
                for h in range(HL):
                    for jq in range(NSB):
                        qd[(b, h, jq)] = dram.tile([128, 512], BF16, name=f"qd{b}{h}{jq}")
                        kd[(b, h, jq)] = dram.tile([128, 512], BF16, name=f"kd{b}{h}{jq}")
                        vd[(b, h, jq)] = dram.tile([128, 512], BF16, name=f"vd{b}{h}{jq}")
            agin, agout = {}, {}
            for b in range(B):
                for jq in range(NSB):
                    agin[(b, jq)] = dram.tile([DH, 512], BF16, name=f"agin{b}{jq}")
                    agout[(b, jq)] = dram.tile([NC * DH, 512], BF16,
                                               addr_space="Shared", name=f"agout{b}{jq}")
            # per-head split for the very last chunk (b=1, jq=0): its gather
            # starts after each head instead of after the whole chunk
            agin_h, agout_h = {}, {}
            for h in range(HL):
                agin_h[h] = dram.tile([128, 512], BF16, name=f"aginh{h}")
                agout_h[h] = dram.tile([NC * 128, 512], BF16,
                                       addr_space="Shared", name=f"agouth{h}")

            # ---- constants ----
            mask_sb = cons.tile([128, 128], F32, name="mask_sb")
            nc.sync.dma_start(mask_sb[:], mask128[:])
            o128_sb = cons.tile([128, 1], BF16, name="o128_sb")
            nc.sync.dma_start(o128_sb[:], ones128[:])

            def load_x(px, t):
                xt = []
                for c in range(NCT):
                    xc = px.tile([128, 512], BF16, tag="xt", bufs=32, name=f"x{t}_{c}")
                    nc.sync.dma_start(xc[:], xS[t, c * 128:(c + 1) * 128, :])
                    xt.append(xc)
                return xt

            def proj_batch(pw, px, prw, pps, b, xt_first):
                for t in range(NSB * b, NSB * (b + 1)):
                    jq = t % NSB
                    off = jq * 512
                    cos_t = prw.tile([128, 512], BF16, tag="cos", name=f"cos{t}")
                    sin_t = prw.tile([128, 512], BF16, tag="sin", name=f"sin{t}")
                    nc.sync.dma_start(cos_t[:], cosE[:, off:off + 512])
                    nc.sync.dma_start(sin_t[:], sinE[:, off:off + 512])
                    xt = xt_first if (t == NSB * b and xt_first is not None) else load_x(px, t)
                    # q/k projections (dim-major) + rope -> DRAM chunk tiles
                    for wname, dst in (("q", qd), ("k", kd)):
                        for i in range(HL):
                            ps = pps.tile([128, 512], F32, tag="pp", bufs=2, name=f"ps{wname}{t}{i}")
                            for c in range(NCT):
                                nc.tensor.matmul(
                                    ps[:], pw[(wname, c)][:, i * 128:(i + 1) * 128],
                                    xt[c][:], start=(c == 0), stop=(c == NCT - 1))
                            qsb = prw.tile([128, 512], BF16, tag="qsb", name=f"qq{wname}{t}{i}")
                            nc.any.tensor_copy(qsb[:], ps[:])
                            # pair-swap via SBUF->SBUF DMA (sign folded into sinE)
                            qsw = prw.tile([128, 512], BF16, tag="qsw", name=f"qw{wname}{t}{i}")
                            qsb_r = qsb.rearrange("(d two) n -> two d n", two=2)
                            qsw_r = qsw.rearrange("(d two) n -> two d n", two=2)
                            nc.sync.dma_start(qsw_r[0], qsb_r[1])
                            nc.sync.dma_start(qsw_r[1], qsb_r[0])
                            qc = prw.tile([128, 512], BF16, tag="qc", name=f"qc{wname}{t}{i}")
                            nc.vector.tensor_tensor(qc[:], ps[:], cos_t[:], mybir.AluOpType.mult)
                            qr = prw.tile([128, 512], BF16, tag="qr", name=f"qr{wname}{t}{i}")
                            nc.vector.tensor_tensor(qr[:], qsw[:], sin_t[:], mybir.AluOpType.mult)
                            qf = prw.tile([128, 512], BF16, tag="qf", name=f"qf{wname}{t}{i}")
                            nc.vector.tensor_tensor(qf[:], qc[:], qr[:], mybir.AluOpType.add)
                            nc.sync.dma_start(dst[(b, i, jq)][:], qf[:])
                    # v projection (token-major), tt-pairs, wv resident
                    for half in range(2):
                        psv = []
                        for u in range(2):
                            p = pps.tile([128, 512], F32, tag="pp", bufs=2, name=f"psv{t}{half}{u}")
                            psv.append(p)
                        for c in range(NCT):
                            for u in range(2):
                                tt = half * 2 + u
                                nc.tensor.matmul(
                                    psv[u][:], xt[c][:, tt * 128:(tt + 1) * 128],
                                    pw[("v", c)][:], start=(c == 0), stop=(c == NCT - 1))
                        for u in range(2):
                            tt = half * 2 + u
                            vst = prw.tile([128, 512], BF16, tag="vst", bufs=2, name=f"vs{t}{half}{u}")
                            nc.any.tensor_copy(vst[:], psv[u][:])
                            for h in range(HL):
                                nc.sync.dma_start(
                                    vd[(b, h, jq)][:, tt * 128:(tt + 1) * 128],
                                    vst[:, h * 128:(h + 1) * 128])

            attn_state = {}

            def attn_setup(b):
                vh, qh, kh = {}, {}, {}
                for h in range(HL):
                    qh[h] = qkh.tile([128, S], BF16, tag="qh", bufs=4, name=f"qh{b}{h}")
                    kh[h] = qkh.tile([128, S], BF16, tag="kh", bufs=4, name=f"kh{b}{h}")
                    vh[h] = vhp.tile([128, NKB * 128], BF16, tag="vh", name=f"vh{b}{h}")
                for jq in range(NSB):
                    sl = slice(jq * 512, (jq + 1) * 512)
                    for h in range(HL):
                        nc.sync.dma_start(qh[h][:, sl], qd[(b, h, jq)][:])
                        nc.sync.dma_start(kh[h][:, sl], kd[(b, h, jq)][:])
                        nc.sync.dma_start(vh[h][:, sl], vd[(b, h, jq)][:])
                attn_state[b] = (qh, kh, vh)

            def attn_chunk(b, jq, per_head_ag=False):
                qh, kh, vh = attn_state[b]
                for h in range(HL):
                    acc = aps.tile([128, 512], F32, tag="acc", bufs=2, name=f"acc{b}{jq}{h}")
                    # exp tiles accumulate on DVE; one sums matmul per block
                    exa = aw.tile([128, 512], BF16, tag="exa", bufs=2, name=f"exa{b}{jq}{h}")
                    nkt = 4 * (jq + 1)
                    for kt in range(nkt):
                        diag = kt >= 4 * jq
                        m = kt - 4 * jq
                        qoff = jq * 512 + (m * 128 if diag else 0)
                        n = 512 - (m * 128 if diag else 0)
                        ro = qoff - jq * 512      # offset within acc/exa
                        pss = aps.tile([128, 512], F32, tag="pss", bufs=3, name=f"pss{b}{jq}{h}{kt}")
                        nc.tensor.matmul(
                            pss[:, :n], kh[h][:, kt * 128:(kt + 1) * 128],
                            qh[h][:, qoff:jq * 512 + 512], start=True, stop=True)
                        if diag:
                            nc.vector.tensor_tensor(
                                pss[:, :128], pss[:, :128], mask_sb[:],
                                mybir.AluOpType.add)
                        if kt == 0:
                            ex = exa           # exp writes the accumulator directly
                        else:
                            ex = aw.tile([128, 512], BF16, tag="ex", bufs=4, name=f"ex{b}{jq}{h}{kt}")
                        nc.scalar.activation(ex[:, :n], pss[:, :n],
                                             mybir.ActivationFunctionType.Exp, scale=SCALE)
                        nc.tensor.matmul(acc[:, ro:512], vh[h][:, kt * 128:(kt + 1) * 128],
                                         ex[:, :n], start=(kt == 0), stop=(kt == nkt - 1))
                        if kt > 0:
                            nc.vector.tensor_tensor(exa[:, ro:512], exa[:, ro:512],
                                                    ex[:, :n], mybir.AluOpType.add)
                    sums = aps.tile([1, 512], F32, tag="sums", bufs=1, name=f"sums{b}{jq}{h}")
                    nc.tensor.matmul(sums[:], o128_sb[:], exa[:], start=True, stop=True)
                    rec = aw.tile([1, 512], BF16, tag="rec", bufs=1, name=f"rec{b}{jq}{h}")
                    with nc.allow_low_precision(reason="softmax denom reciprocal in bf16 is fine at 2e-2 tol"):
                        nc.vector.reciprocal(rec[:], sums[:])
                    rbs = aw.tile([128, 512], BF16, tag="rbs", bufs=2, name=f"rbs{b}{jq}{h}")
                    nc.gpsimd.partition_broadcast(rbs[:], rec[:])
                    att = aw.tile([128, 512], BF16, tag="att", bufs=2, name=f"att{b}{jq}{h}")
                    nc.vector.tensor_tensor(att[:], acc[:], rbs[:], mybir.AluOpType.mult)
                    if per_head_ag:
                        nc.sync.dma_start(agin_h[h][:], att[:])
                        nc.gpsimd.collective_compute(
                            "AllGather", mybir.AluOpType.bypass,
                            replica_groups=[list(range(NC))],
                            ins=[agin_h[h].opt()], outs=[agout_h[h].opt()])
                    else:
                        nc.sync.dma_start(agin[(b, jq)][h * 128:(h + 1) * 128, :], att[:])
                if not per_head_ag:
                    nc.gpsimd.collective_compute(
                        "AllGather", mybir.AluOpType.bypass,
                        replica_groups=[list(range(NC))],
                        ins=[agin[(b, jq)].opt()], outs=[agout[(b, jq)].opt()])

            # ================= emission =================
            with tc.tile_pool(name="pw", bufs=1) as pwp, \
                 tc.tile_pool(name="px", bufs=1) as px, \
                 tc.tile_pool(name="prw", bufs=2) as prw, \
                 tc.tile_pool(name="pps", bufs=1, space="PSUM") as pps:
                # interleave x slice 0 with wq: first psum group's deps land first
                pw = {}
                xt0 = []
                for c in range(NCT):
                    xc = px.tile([128, 512], BF16, tag="xt", bufs=32, name=f"x0_{c}")
                    nc.sync.dma_start(xc[:], xS[0, c * 128:(c + 1) * 128, :])
                    xt0.append(xc)
                    wt = pwp.tile([128, DH], BF16, name=f"wq_{c}")
                    nc.sync.dma_start(wt[:], wqT[c * 128:(c + 1) * 128, :])
                    pw[("q", c)] = wt
                for wname, w_dr in (("k", wkT), ("v", wvT)):
                    for c in range(NCT):
                        wt = pwp.tile([128, DH], BF16, name=f"w{wname}_{c}")
                        nc.sync.dma_start(wt[:], w_dr[c * 128:(c + 1) * 128, :])
                        pw[(wname, c)] = wt
                proj_batch(pw, px, prw, pps, 0, xt0)
                attn_setup(0)
                for jq in range(NSB):
                    attn_chunk(0, jq)
                proj_batch(pw, px, prw, pps, 1, None)
            # projection pools (weights, x, rope work, proj psum) released here

            with tc.tile_pool(name="ores", bufs=1) as ores, \
                 tc.tile_pool(name="och", bufs=1) as och, \
                 tc.tile_pool(name="oo", bufs=4) as oo, \
                 tc.tile_pool(name="ops", bufs=2, space="PSUM") as ops:
                wo_sb = ores.tile([128, NCT * DH], BF16, name="wo_sb")
                nc.sync.dma_start(
                    wo_sb[:].rearrange("p (c i) -> p c i", c=NCT),
                    woT.rearrange("(c p) i -> p c i", p=128))

                def oproj_chunk(b, jq):
                    # contiguous per-feature-block loads of the gathered chunk
                    ch = []
                    for c in range(NCT):
                        cc = och.tile([128, 512], BF16, tag="ch", bufs=40, name=f"ch{b}{jq}{c}")
                        nc.sync.dma_start(cc[:], agout[(b, jq)][c * 128:(c + 1) * 128, :])
                        ch.append(cc)
                    for tt in range(4):
                        pso = ops.tile([128, 512], F32, tag="pso", name=f"pso{b}{jq}{tt}")
                        for c in range(NCT):
                            nc.tensor.matmul(pso[:], ch[c][:, tt * 128:(tt + 1) * 128],
                                             wo_sb[:, c * DH:(c + 1) * DH],
                                             start=(c == 0), stop=(c == NCT - 1))
                        ot = oo.tile([128, 512], F32, tag="ot", name=f"ot{b}{jq}{tt}")
                        nc.any.tensor_copy(ot[:], pso[:])
                        row = b * S + jq * 512 + tt * 128
                        nc.sync.dma_start(out[row:row + 128, :], ot[:])

                def oproj_chunk_h():
                    # final chunk (b=1, jq=0), gathered per head: contraction
                    # grouped h-major so early matmuls only need early gathers
                    ch = {}
                    for h in range(HL):
                        for r in range(NC):
                            f = r * HL + h
                            cc = och.tile([128, 512], BF16, tag="ch", bufs=40, name=f"chh{f}")
                            nc.sync.dma_start(cc[:], agout_h[h][r * 128:(r + 1) * 128, :])
                            ch[f] = cc
                    for tt in range(4):
                        pso = ops.tile([128, 512], F32, tag="pso", name=f"psoh{tt}")
                        for ci, (h, r) in enumerate((h, r) for h in range(HL) for r in range(NC)):
                            f = r * HL + h
                            nc.tensor.matmul(pso[:], ch[f][:, tt * 128:(tt + 1) * 128],
                                             wo_sb[:, f * DH:(f + 1) * DH],
                                             start=(ci == 0), stop=(ci == NCT - 1))
                        ot = oo.tile([128, 512], F32, tag="ot", name=f"oth{tt}")
                        nc.any.tensor_copy(ot[:], pso[:])
                        row = S + tt * 128
                        nc.sync.dma_start(out[row:row + 128, :], ot[:])

                # attention b1 (chunk order puts the smallest chunk last, with
                # per-head gathers) interleaved with o_proj of batch 0's chunks
                attn_setup(1)
                for idx, jq in enumerate((1, 2, 3, 0)):
                    attn_chunk(1, jq, per_head_ag=(jq == 0))
                    oproj_chunk(0, idx)
                for jq in (1, 2, 3):
                    oproj_chunk(1, jq)
                oproj_chunk_h()

    nc.compile()
    return nc


def _host_prep(x, freqs_cos, freqs_sin, mask, wq, wk, wv, wo):
    xT = np.asarray(x, np.float32).reshape(T, D).T        # [D, T]
    xSm = np.ascontiguousarray(
        xT.reshape(D, NT, 512).transpose(1, 0, 2)).astype(bf16)   # [NT, D, 512]
    cos = np.asarray(freqs_cos, np.float32)   # [S, 64]
    sin = np.asarray(freqs_sin, np.float32)
    cosE = np.ascontiguousarray(np.repeat(cos.T, 2, axis=0)).astype(bf16)  # [128, S]
    sinE = np.repeat(sin.T, 2, axis=0)                     # [128, S]
    sinE[0::2, :] *= -1.0                                  # sign fold: even rows negative
    sinE = np.ascontiguousarray(sinE).astype(bf16)
    # causal triangle for a 128x128 diagonal block, pre-scaled for exp(scale*x)
    kk = np.arange(128)
    mask128 = np.where(kk[:, None] > kk[None, :], -1e9 / SCALE, 0.0).astype(np.float32)
    ones128 = np.ones((128, 1), bf16)
    shared = dict(xS=xSm, cosE=cosE, sinE=sinE, mask128=mask128, ones128=ones128)
    in_maps = []
    for r in range(NC):
        sl = slice(r * DH, (r + 1) * DH)
        m = dict(shared)
        m["wqT"] = np.ascontiguousarray(np.asarray(wq, np.float32)[sl, :].T).astype(bf16)
        m["wkT"] = np.ascontiguousarray(np.asarray(wk, np.float32)[sl, :].T).astype(bf16)
        m["wvT"] = np.ascontiguousarray(np.asarray(wv, np.float32)[sl, :].T).astype(bf16)
        m["woT"] = np.ascontiguousarray(np.asarray(wo, np.float32)[sl, :].T).astype(bf16)
        in_maps.append(m)
    return in_maps


def kernel(x, freqs_cos, freqs_sin, mask, wq, wk, wv, wo, start_pos):
    global LAST_RESULT
    if "nc" not in _CACHE:
        _CACHE["nc"] = build()
    nc = _CACHE["nc"]
    in_maps = _host_prep(x, freqs_cos, freqs_sin, mask, wq, wk, wv, wo)
    res = run_bass_kernel_spmd(nc, in_maps, core_ids=list(range(NC)))
    LAST_RESULT = res
    parts = [res.results[r]["out"] for r in range(NC)]
    full = np.concatenate(parts, axis=1)      # [T, D]
    return np.ascontiguousarray(full.reshape(B, S, D)).astype(np.float32)


# revision 32
# speedup vs baseline: 1.5414x; 1.0055x over previous
"""Trainium2 8-core tensor-parallel attention kernel (Bass/Tile).

Full inputs in, full output out. Tensor-parallel over heads (4 per core).
Fused schedule: proj(b0) -> attn(b0) || proj(b1) -> attn(b1) || o_proj,
with one AllGather per (batch, 512-query chunk) overlapped with compute,
and o_proj chunks emission-interleaved with attention.
Causal structure exploited at 128-column granularity on the diagonal.
All hot DMA paths are fully contiguous (slice-major x on the host,
per-(batch,head,chunk) q/k/v DRAM tiles, per-feature-block o_proj loads).
RoPE rotation done by a partition pair-swap DMA with the sign folded
into sin (no tensor-engine rotation matmuls).
"""
import sys

for _p in ("/opt/trn_rl_repo",):
    if _p not in sys.path:
        sys.path.insert(0, _p)

import numpy as np
import ml_dtypes

import concourse.bass as bass
import concourse.mybir as mybir
import concourse.tile as tile
from concourse import bacc
from concourse.bass_utils import run_bass_kernel_spmd

B, S, D, H = 2, 2048, 4096, 32
HD = D // H          # 128 head dim
T = B * S            # 4096 tokens
NC = 8               # cores
HL = H // NC         # 4 heads per core
DH = HL * HD         # 512 dims per core
SCALE = 1.0 / float(np.sqrt(HD))
BF16 = mybir.dt.bfloat16
F32 = mybir.dt.float32
bf16 = ml_dtypes.bfloat16

NT = T // 512        # 8 token slices of 512
NSB = S // 512       # 4 slices per batch
NCT = D // 128       # 32 contraction tiles
NKB = S // 128       # 16 key blocks per batch

_CACHE = {}
LAST_RESULT = None


def build():
    nc = bacc.Bacc("TRN2", target_bir_lowering=False, debug=False, num_devices=NC)

    # x stored slice-major: [NT, D, 512] so every x tile DMA is contiguous
    xS = nc.dram_tensor("xS", [NT, D, 512], BF16, kind="ExternalInput").ap()
    wqT = nc.dram_tensor("wqT", [D, DH], BF16, kind="ExternalInput").ap()
    wkT = nc.dram_tensor("wkT", [D, DH], BF16, kind="ExternalInput").ap()
    wvT = nc.dram_tensor("wvT", [D, DH], BF16, kind="ExternalInput").ap()
    woT = nc.dram_tensor("woT", [D, DH], BF16, kind="ExternalInput").ap()
    cosE = nc.dram_tensor("cosE", [HD, S], BF16, kind="ExternalInput").ap()
    sinE = nc.dram_tensor("sinE", [HD, S], BF16, kind="ExternalInput").ap()  # sign-folded
    mask128 = nc.dram_tensor("mask128", [128, 128], F32, kind="ExternalInput").ap()
    ones128 = nc.dram_tensor("ones128", [128, 1], BF16, kind="ExternalInput").ap()
    out = nc.dram_tensor("out", [T, DH], F32, kind="ExternalOutput").ap()

    with tile.TileContext(nc) as tc:
        with tc.tile_pool(name="dram", bufs=1, space="DRAM") as dram, \
             tc.tile_pool(name="cons", bufs=1) as cons, \
             tc.tile_pool(name="qkh", bufs=1) as qkh, \
             tc.tile_pool(name="vh", bufs=5) as vhp, \
             tc.tile_pool(name="aw", bufs=1) as aw, \
             tc.tile_pool(name="aps", bufs=1, space="PSUM") as aps:

            # ---- DRAM internals: per (b, head, jq-chunk), all contiguous ----
            qd, kd, vd = {}, {}, {}
            for b in range(B):
                for h in range(HL):
                    for jq in range(NSB):
                        qd[(b, h, jq)] = dram.tile([128, 512], BF16, name=f"qd{b}{h}{jq}")
                        kd[(b, h, jq)] = dram.tile([128, 512], BF16, name=f"kd{b}{h}{jq}")
                        vd[(b, h, jq)] = dram.tile([128, 512], BF16, name=f"vd{b}{h}{jq}")
            agin, agout = {}, {}
            for b in range(B):
                for jq in range(NSB):
                    agin[(b, jq)] = dram.tile([DH, 512], BF16, name=f"agin{b}{jq}")
                    agout[(b, jq)] = dram.tile([NC * DH, 512], BF16,
                                               addr_space="Shared", name=f"agout{b}{jq}")
            # per-head split for the very last chunk (b=1, jq=0): its gather
            # starts after each head instead of after the whole chunk
            agin_h, agout_h = {}, {}
            for h in range(HL):
                agin_h[h] = dram.tile([128, 512], BF16, name=f"aginh{h}")
                agout_h[h] = dram.tile([NC * 128, 512], BF16,
                                       addr_space="Shared", name=f"agouth{h}")

            # ---- constants ----
            mask_sb = cons.tile([128, 128], F32, name="mask_sb")
            nc.sync.dma_start(mask_sb[:], mask128[:])
            o128_sb = cons.tile([128, 1], BF16, name="o128_sb")
            nc.sync.dma_start(o128_sb[:], ones128[:])

            def load_x(px, t):
                xt = []
                for c in range(NCT):
                    xc = px.tile([128, 512], BF16, tag="xt", bufs=32, name=f"x{t}_{c}")
                    nc.sync.dma_start(xc[:], xS[t, c * 128:(c + 1) * 128, :])
                    xt.append(xc)
                return xt

            def proj_slice(pw, px, prw, pps, t, xt_first=None, split_first=False):
                    b = t // NSB
                    jq = t % NSB
                    off = jq * 512
                    cos_t = prw.tile([128, 512], BF16, tag="cos", name=f"cos{t}")
                    sin_t = prw.tile([128, 512], BF16, tag="sin", name=f"sin{t}")
                    nc.sync.dma_start(cos_t[:], cosE[:, off:off + 512])
                    nc.sync.dma_start(sin_t[:], sinE[:, off:off + 512])
                    xt = xt_first if xt_first is not None else load_x(px, t)

                    def rope_drain(wname, i, ps):
                            dst = qd if wname == "q" else kd
                            qsb = prw.tile([128, 512], BF16, tag="qsb", name=f"qq{wname}{t}{i}")
                            nc.any.tensor_copy(qsb[:], ps[:])
                            # pair-swap via SBUF->SBUF DMA (sign folded into sinE)
                            qsw = prw.tile([128, 512], BF16, tag="qsw", name=f"qw{wname}{t}{i}")
                            qsb_r = qsb.rearrange("(d two) n -> two d n", two=2)
                            qsw_r = qsw.rearrange("(d two) n -> two d n", two=2)
                            nc.sync.dma_start(qsw_r[0], qsb_r[1])
                            nc.sync.dma_start(qsw_r[1], qsb_r[0])
                            qc = prw.tile([128, 512], BF16, tag="qc", name=f"qc{wname}{t}{i}")
                            nc.vector.tensor_tensor(qc[:], ps[:], cos_t[:], mybir.AluOpType.mult)
                            qr = prw.tile([128, 512], BF16, tag="qr", name=f"qr{wname}{t}{i}")
                            nc.vector.tensor_tensor(qr[:], qsw[:], sin_t[:], mybir.AluOpType.mult)
                            qf = prw.tile([128, 512], BF16, tag="qf", name=f"qf{wname}{t}{i}")
                            nc.vector.tensor_tensor(qf[:], qc[:], qr[:], mybir.AluOpType.add)
                            nc.sync.dma_start(dst[(b, i, jq)][:], qf[:])

                    if split_first:
                        # slice 0: pair up q psum groups, half contraction each,
                        # so the PE starts before the full 8MB of x+wq lands
                        ps_open = {}
                        for i, ha in [(0, 0), (1, 0), (0, 1), (2, 0), (1, 1),
                                      (3, 0), (2, 1), (3, 1)]:
                            if ha == 0:
                                ps_open[i] = pps.tile([128, 512], F32, tag="pp", bufs=2,
                                                      name=f"psq{t}{i}")
                            ps = ps_open[i]
                            for c in range(16 * ha, 16 * ha + 16):
                                nc.tensor.matmul(
                                    ps[:], pw[("q", c)][:, i * 128:(i + 1) * 128],
                                    xt[c][:], start=(c == 0), stop=(c == NCT - 1))
                            if ha == 1:
                                rope_drain("q", i, ps)
                        qk_groups = [("k", i) for i in range(HL)]
                    else:
                        qk_groups = [("q", i) for i in range(HL)] + [("k", i) for i in range(HL)]

                    for wname, i in qk_groups:
                        ps = pps.tile([128, 512], F32, tag="pp", bufs=2, name=f"ps{wname}{t}{i}")
                        for c in range(NCT):
                            nc.tensor.matmul(
                                ps[:], pw[(wname, c)][:, i * 128:(i + 1) * 128],
                                xt[c][:], start=(c == 0), stop=(c == NCT - 1))
                        rope_drain(wname, i, ps)
                    # v projection (token-major), tt-pairs, wv resident
                    for half in range(2):
                        psv = []
                        for u in range(2):
                            p = pps.tile([128, 512], F32, tag="pp", bufs=2, name=f"psv{t}{half}{u}")
                            psv.append(p)
                        for c in range(NCT):
                            for u in range(2):
                                tt = half * 2 + u
                                nc.tensor.matmul(
                                    psv[u][:], xt[c][:, tt * 128:(tt + 1) * 128],
                                    pw[("v", c)][:], start=(c == 0), stop=(c == NCT - 1))
                        for u in range(2):
                            tt = half * 2 + u
                            vst = prw.tile([128, 512], BF16, tag="vst", bufs=2, name=f"vs{t}{half}{u}")
                            nc.any.tensor_copy(vst[:], psv[u][:])
                            for h in range(HL):
                                nc.sync.dma_start(
                                    vd[(b, h, jq)][:, tt * 128:(tt + 1) * 128],
                                    vst[:, h * 128:(h + 1) * 128])

            attn_state = {}

            def attn_setup_alloc(b):
                vh, qh, kh = {}, {}, {}
                for h in range(HL):
                    qh[h] = qkh.tile([128, S], BF16, tag="qh", bufs=4, name=f"qh{b}{h}")
                    kh[h] = qkh.tile([128, S], BF16, tag="kh", bufs=4, name=f"kh{b}{h}")
                    vh[h] = vhp.tile([128, NKB * 128], BF16, tag="vh", name=f"vh{b}{h}")
                attn_state[b] = (qh, kh, vh)

            def attn_load_chunk(b, jq):
                # MUST be emitted after proj slice (b, jq) so the DRAM reads
                # order after the writes
                qh, kh, vh = attn_state[b]
                sl = slice(jq * 512, (jq + 1) * 512)
                for h in range(HL):
                    nc.sync.dma_start(qh[h][:, sl], qd[(b, h, jq)][:])
                    nc.sync.dma_start(kh[h][:, sl], kd[(b, h, jq)][:])
                    nc.sync.dma_start(vh[h][:, sl], vd[(b, h, jq)][:])

            def attn_chunk(b, jq, per_head_ag=False):
                qh, kh, vh = attn_state[b]
                for h in range(HL):
                    acc = aps.tile([128, 512], F32, tag="acc", bufs=2, name=f"acc{b}{jq}{h}")
                    # exp tiles accumulate on DVE; one sums matmul per block
                    exa = aw.tile([128, 512], BF16, tag="exa", bufs=2, name=f"exa{b}{jq}{h}")
                    nkt = 4 * (jq + 1)
                    for kt in range(nkt):
                        diag = kt >= 4 * jq
                        m = kt - 4 * jq
                        qoff = jq * 512 + (m * 128 if diag else 0)
                        n = 512 - (m * 128 if diag else 0)
                        ro = qoff - jq * 512      # offset within acc/exa
                        pss = aps.tile([128, 512], F32, tag="pss", bufs=3, name=f"pss{b}{jq}{h}{kt}")
                        nc.tensor.matmul(
                            pss[:, :n], kh[h][:, kt * 128:(kt + 1) * 128],
                            qh[h][:, qoff:jq * 512 + 512], start=True, stop=True)
                        if diag:
                            nc.vector.tensor_tensor(
                                pss[:, :128], pss[:, :128], mask_sb[:],
                                mybir.AluOpType.add)
                        if kt == 0:
                            ex = exa           # exp writes the accumulator directly
                        else:
                            ex = aw.tile([128, 512], BF16, tag="ex", bufs=4, name=f"ex{b}{jq}{h}{kt}")
                        nc.scalar.activation(ex[:, :n], pss[:, :n],
                                             mybir.ActivationFunctionType.Exp, scale=SCALE)
                        nc.tensor.matmul(acc[:, ro:512], vh[h][:, kt * 128:(kt + 1) * 128],
                                         ex[:, :n], start=(kt == 0), stop=(kt == nkt - 1))
                        if kt > 0:
                            nc.vector.tensor_tensor(exa[:, ro:512], exa[:, ro:512],
                                                    ex[:, :n], mybir.AluOpType.add)
                    sums = aps.tile([1, 512], F32, tag="sums", bufs=1, name=f"sums{b}{jq}{h}")
                    nc.tensor.matmul(sums[:], o128_sb[:], exa[:], start=True, stop=True)
                    rec = aw.tile([1, 512], BF16, tag="rec", bufs=1, name=f"rec{b}{jq}{h}")
                    with nc.allow_low_precision(reason="softmax denom reciprocal in bf16 is fine at 2e-2 tol"):
                        nc.vector.reciprocal(rec[:], sums[:])
                    rbs = aw.tile([128, 512], BF16, tag="rbs", bufs=2, name=f"rbs{b}{jq}{h}")
                    nc.gpsimd.partition_broadcast(rbs[:], rec[:])
                    att = aw.tile([128, 512], BF16, tag="att", bufs=2, name=f"att{b}{jq}{h}")
                    nc.vector.tensor_tensor(att[:], acc[:], rbs[:], mybir.AluOpType.mult)
                    if per_head_ag:
                        nc.sync.dma_start(agin_h[h][:], att[:])
                        nc.gpsimd.collective_compute(
                            "AllGather", mybir.AluOpType.bypass,
                            replica_groups=[list(range(NC))],
                            ins=[agin_h[h].opt()], outs=[agout_h[h].opt()])
                    else:
                        nc.sync.dma_start(agin[(b, jq)][h * 128:(h + 1) * 128, :], att[:])
                if not per_head_ag:
                    nc.gpsimd.collective_compute(
                        "AllGather", mybir.AluOpType.bypass,
                        replica_groups=[list(range(NC))],
                        ins=[agin[(b, jq)].opt()], outs=[agout[(b, jq)].opt()])

            # ================= emission =================
            with tc.tile_pool(name="pw", bufs=1) as pwp, \
                 tc.tile_pool(name="px", bufs=1) as px, \
                 tc.tile_pool(name="prw", bufs=2) as prw, \
                 tc.tile_pool(name="pps", bufs=1, space="PSUM") as pps:
                # interleave x slice 0 with wq: first psum group's deps land first
                pw = {}
                xt0 = []
                for c in range(NCT):
                    xc = px.tile([128, 512], BF16, tag="xt", bufs=32, name=f"x0_{c}")
                    nc.sync.dma_start(xc[:], xS[0, c * 128:(c + 1) * 128, :])
                    xt0.append(xc)
                    wt = pwp.tile([128, DH], BF16, name=f"wq_{c}")
                    nc.sync.dma_start(wt[:], wqT[c * 128:(c + 1) * 128, :])
                    pw[("q", c)] = wt
                for wname, w_dr in (("k", wkT), ("v", wvT)):
                    for c in range(NCT):
                        wt = pwp.tile([128, DH], BF16, name=f"w{wname}_{c}")
                        nc.sync.dma_start(wt[:], w_dr[c * 128:(c + 1) * 128, :])
                        pw[(wname, c)] = wt
                # attention chunks interleave into projection emission as soon
                # as their dependency slices are written -> earlier gathers
                proj_slice(pw, px, prw, pps, 0, xt_first=xt0, split_first=True)
                attn_setup_alloc(0)
                attn_load_chunk(0, 0)
                proj_slice(pw, px, prw, pps, 1)
                attn_load_chunk(0, 1)
                attn_chunk(0, 0)
                proj_slice(pw, px, prw, pps, 2)
                attn_load_chunk(0, 2)
                attn_chunk(0, 1)
                proj_slice(pw, px, prw, pps, 3)
                attn_load_chunk(0, 3)
                attn_chunk(0, 2)
                proj_slice(pw, px, prw, pps, 4)
                attn_setup_alloc(1)
                attn_load_chunk(1, 0)
                attn_chunk(0, 3)
                proj_slice(pw, px, prw, pps, 5)
                attn_load_chunk(1, 1)
                attn_chunk(1, 1)
                proj_slice(pw, px, prw, pps, 6)
                attn_load_chunk(1, 2)
                attn_chunk(1, 2)
                proj_slice(pw, px, prw, pps, 7)
                attn_load_chunk(1, 3)
                attn_chunk(1, 3)
                attn_chunk(1, 0, per_head_ag=True)
            # projection pools (weights, x, rope work, proj psum) released here

            with tc.tile_pool(name="ores", bufs=1) as ores, \
                 tc.tile_pool(name="och", bufs=1) as och, \
                 tc.tile_pool(name="oo", bufs=4) as oo, \
                 tc.tile_pool(name="ops", bufs=2, space="PSUM") as ops:
                wo_sb = ores.tile([128, NCT * DH], BF16, name="wo_sb")
                nc.sync.dma_start(
                    wo_sb[:].rearrange("p (c i) -> p c i", c=NCT),
                    woT.rearrange("(c p) i -> p c i", p=128))

                def oproj_chunk(b, jq):
                    # contiguous per-feature-block loads of the gathered chunk
                    ch = []
                    for c in range(NCT):
                        cc = och.tile([128, 512], BF16, tag="ch", bufs=40, name=f"ch{b}{jq}{c}")
                        nc.sync.dma_start(cc[:], agout[(b, jq)][c * 128:(c + 1) * 128, :])
                        ch.append(cc)
                    for tt in range(4):
                        pso = ops.tile([128, 512], F32, tag="pso", name=f"pso{b}{jq}{tt}")
                        for c in range(NCT):
                            nc.tensor.matmul(pso[:], ch[c][:, tt * 128:(tt + 1) * 128],
                                             wo_sb[:, c * DH:(c + 1) * DH],
                                             start=(c == 0), stop=(c == NCT - 1))
                        ot = oo.tile([128, 512], F32, tag="ot", name=f"ot{b}{jq}{tt}")
                        nc.any.tensor_copy(ot[:], pso[:])
                        row = b * S + jq * 512 + tt * 128
                        nc.sync.dma_start(out[row:row + 128, :], ot[:])

                def oproj_chunk_h():
                    # final chunk (b=1, jq=0), gathered per head: contraction
                    # grouped h-major so early matmuls only need early gathers
                    ch = {}
                    for h in range(HL):
                        for r in range(NC):
                            f = r * HL + h
                            cc = och.tile([128, 512], BF16, tag="ch", bufs=40, name=f"chh{f}")
                            nc.sync.dma_start(cc[:], agout_h[h][r * 128:(r + 1) * 128, :])
                            ch[f] = cc
                    for tt in range(4):
                        pso = ops.tile([128, 512], F32, tag="pso", name=f"psoh{tt}")
                        for ci, (h, r) in enumerate((h, r) for h in range(HL) for r in range(NC)):
                            f = r * HL + h
                            nc.tensor.matmul(pso[:], ch[f][:, tt * 128:(tt + 1) * 128],
                                             wo_sb[:, f * DH:(f + 1) * DH],
                                             start=(ci == 0), stop=(ci == NCT - 1))
                        ot = oo.tile([128, 512], F32, tag="ot", name=f"oth{tt}")
                        nc.any.tensor_copy(ot[:], pso[:])
                        row = S + tt * 128
                        nc.sync.dma_start(out[row:row + 128, :], ot[:])

                for jq in range(NSB):
                    oproj_chunk(0, jq)
                for jq in (1, 2, 3):
                    oproj_chunk(1, jq)
                oproj_chunk_h()

    nc.compile()
    return nc


def _host_prep(x, freqs_cos, freqs_sin, mask, wq, wk, wv, wo):
    xT = np.asarray(x, np.float32).reshape(T, D).T        # [D, T]
    xSm = np.ascontiguousarray(
        xT.reshape(D, NT, 512).transpose(1, 0, 2)).astype(bf16)   # [NT, D, 512]
    cos = np.asarray(freqs_cos, np.float32)   # [S, 64]
    sin = np.asarray(freqs_sin, np.float32)
    cosE = np.ascontiguousarray(np.repeat(cos.T, 2, axis=0)).astype(bf16)  # [128, S]
    sinE = np.repeat(sin.T, 2, axis=0)                     # [128, S]
    sinE[0::2, :] *= -1.0                                  # sign fold: even rows negative
    sinE = np.ascontiguousarray(sinE).astype(bf16)
    # causal triangle for a 128x128 diagonal block, pre-scaled for exp(scale*x)
    kk = np.arange(128)
    mask128 = np.where(kk[:, None] > kk[None, :], -1e9 / SCALE, 0.0).astype(np.float32)
    ones128 = np.ones((128, 1), bf16)
    shared = dict(xS=xSm, cosE=cosE, sinE=sinE, mask128=mask128, ones128=ones128)
    in_maps = []
    for r in range(NC):
        sl = slice(r * DH, (r + 1) * DH)
        m = dict(shared)
        m["wqT"] = np.ascontiguousarray(np.asarray(wq, np.float32)[sl, :].T).astype(bf16)
        m["wkT"] = np.ascontiguousarray(np.asarray(wk, np.float32)[sl, :].T).astype(bf16)
        m["wvT"] = np.ascontiguousarray(np.asarray(wv, np.float32)[sl, :].T).astype(bf16)
        m["woT"] = np.ascontiguousarray(np.asarray(wo, np.float32)[sl, :].T).astype(bf16)
        in_maps.append(m)
    return in_maps


def kernel(x, freqs_cos, freqs_sin, mask, wq, wk, wv, wo, start_pos):
    global LAST_RESULT
    if "nc" not in _CACHE:
        _CACHE["nc"] = build()
    nc = _CACHE["nc"]
    in_maps = _host_prep(x, freqs_cos, freqs_sin, mask, wq, wk, wv, wo)
    res = run_bass_kernel_spmd(nc, in_maps, core_ids=list(range(NC)))
    LAST_RESULT = res
    parts = [res.results[r]["out"] for r in range(NC)]
    full = np.concatenate(parts, axis=1)      # [T, D]
    return np.ascontiguousarray(full.reshape(B, S, D)).astype(np.float32)
